# revision 9
# baseline (speedup 1.0000x reference)
"""ACT universal-transformer encoder (nn_Encoder_38165079392904) on 8 TRN2 cores.

Strategy: pure data-parallel over batch (B=8 -> 1 batch element per core, no
collectives). Per core, activations are kept in "transposed" layout
[feature_dim on partitions, sequence on free], so every GEMM is a natural
Trainium matmul (out = lhsT.T @ rhs, contracting over partitions):

  stT [D=512, S=1024]:  qT,kT = W.T @ stT (transposed); v = stT.T @ Wv (natural)
  logitsT[k,q] = kT_h.T @ qT_h    per head (K=dh=64, two heads share PE rows)
  attnT = exp(logitsT/8)          ACT engine, psum -> fp16
  sums  = ones.T @ attnT          matmul ones-trick (col-paired head pairs)
  ctxT  = v_h.T @ attnT           col-paired head pairs in one psum
  normalization / LN stat rows are broadcast across partitions with K=1
  matmuls; LayerNorm over dff is folded into w2 host-side (w2g = w2*lnf_g).
  ACT halting runs in fp32 on a [128, 8] tile-major layout; the update
  weight row is re-ordered to sequence order by an SBUF->SBUF DMA.

Matmul operands are fp16 (this toolchain's walrus allows only ONE semaphore
wait per ISA instruction; fp16 matmuls lower to LDWEIGHTS+MATMUL and get two
slots, while 4-byte fp32/fp32r matmuls are single self-loading instructions
and routinely fail codegen under Tile). PSUM accumulation is fp32; the
cross-step state st and halting probabilities are fp32.

A post-pass (split_excess_waits) hoists any wait beyond the per-instruction
budget into standalone EventSemaphore instructions on the same engine.
"""

import sys

sys.path.insert(0, "/opt/trn_rl_repo")

import numpy as np
import concourse.bass as bass
import concourse.tile as tile
from concourse import mybir
from concourse.bass_utils import run_bass_kernel_spmd

B, S, D, H, DFF, STEPS = 8, 1024, 512, 8, 2048, 6
DH = D // H          # 64
DT = D // 128        # 4 d-tiles
FT = DFF // 128      # 16 f-tiles
ST = S // 128        # 8 s-tiles
NC = 2               # s-chunks
CH = S // NC         # 512
EPS = 1e-6
THR = 1.0 - 0.01

F32 = mybir.dt.float32
F16 = mybir.dt.float16
AF = mybir.ActivationFunctionType
OP = mybir.AluOpType

N_CORES = 8


def split_excess_waits(nc):
    """Walrus codegen allows one sem-wait per ISA instruction (two for 2-byte
    matmuls via the LDWEIGHTS+MATMUL split). Hoist excess waits into
    standalone EventSemaphore instructions on the same engine, just before
    the instruction, preserving program order and semantics."""
    cnt = 0
    for fn in nc.m.functions:
        for bb in fn.blocks:
            il = bb.instructions
            i = 0
            while i < len(il):
                inst = il[i]
                si = inst.sync_info
                if si is not None and si.on_wait:
                    waits = list(si.on_wait)
                    cap = 1
                    if len(waits) > cap:
                        keep, extra = waits[-cap:], waits[:-cap]
                        for w in extra:
                            ni = mybir.InstEventSemaphore(
                                name=f"{inst.name}-xw{cnt}", ins=[], outs=[],
                                sync_info=mybir.SyncInfo(on_wait=[w], on_update=[]))
                            cnt += 1
                            ni.engine = inst.engine
                            il.insert(i, ni)
                            i += 1
                        inst.sync_info = mybir.SyncInfo(
                            on_wait=keep, on_update=list(si.on_update))
                i += 1
    return cnt


def _build():
    nc = bass.Bass()
    dp = lambda name, shape, dt=F16: nc.declare_dram_parameter(
        name, shape, dt, isOutput=False)

    x_d = dp("x32", [S, D], F32)
    wq_d, wk_d, wv_d, wo_d = (dp(n, [D, D]) for n in ("wq16", "wk16", "wv16", "wo16"))
    w1_d = dp("w116", [D, DFF])
    w2g_d = dp("w2g16", [DFF, D])
    bq_d, bk_d, bo_d = (dp(n, [128, DT], F32) for n in ("bq32", "bk32", "bo32"))
    b1_d = dp("b132", [128, FT], F32)
    bvb_d = dp("bvb32", [128, DT], F32)     # w2.T@lnf_b + b2  (FFN2 combined bias)
    b1l_d = dp("b1ln32", [128, DT], F32)    # ln1 beta as per-partition cols
    b2l_d = dp("b2ln32", [128, DT], F32)    # ln2 beta
    bvr_d = dp("bv16r", [1, D])             # v-proj bias as a row (for bcast)
    vgn_d = dp("vgn16", [1, D])             # -colsum_f(w2g)
    g1p_d = dp("g1p16", [1, D])             # +ln1 gamma (A-term lhsT rows)
    g1n_d = dp("g1n16", [1, D])             # -ln1 gamma (C-term lhsT rows)
    g2p_d = dp("g2p16", [1, D])
    g2n_d = dp("g2n16", [1, D])
    actw_d = dp("actw16", [128, DT])
    actb_d = dp("actb32", [128, 1], F32)
    onesc_d = dp("ones_col16", [128, 1])
    onesr_d = dp("ones_row16", [1, 128])
    ehA_d = dp("ehA16", [1, 128])           # ones in cols 0:64
    ehB_d = dp("ehB16", [1, 128])           # ones in cols 64:128
    eye_d = dp("eye16", [128, 128])
    out_d = nc.declare_dram_parameter("out32", [S, D], F32, isOutput=True)
    uw_scr = nc.dram_tensor("uw_scratch", [128, ST], F16)

    with tile.TileContext(nc) as tc, nc.allow_low_precision(
            reason="fp16 operand pipeline by design; fp32 accumulation in PSUM"):
        with (
            tc.tile_pool(name="persist", bufs=1) as pp,
            tc.tile_pool(name="evac", bufs=6) as pe,        # f32 [128,CH] temps
            tc.tile_pool(name="sqa", bufs=6) as psqa,       # ACT-written fp16 squares
            tc.tile_pool(name="sqv", bufs=6) as psqv,       # DVE fp16 temps
            tc.tile_pool(name="attn", bufs=16) as pa,       # attnT fp16 [128,CH]
            tc.tile_pool(name="rtm", bufs=10) as prt,       # [128,8] halting temps
            tc.tile_pool(name="crow", bufs=4) as pcr,       # [1,CH] fp16 mm-rhs rows
            tc.tile_pool(name="ps_mm", bufs=4, space="PSUM") as ps_mm,
            tc.tile_pool(name="ps_bc", bufs=2, space="PSUM") as ps_bc,
            tc.tile_pool(name="ps_sm", bufs=2, space="PSUM") as ps_sm,
        ):
            # ---------------- persistent tiles + param load ----------------
            def load(name, shape, src, dt=F16):
                t = pp.tile(shape, dt, name=name)
                nc.sync.dma_start(out=t, in_=src)
                return t

            wq = [load(f"wq{k}", [128, D], wq_d[k * 128:(k + 1) * 128, :]) for k in range(DT)]
            wk = [load(f"wk{k}", [128, D], wk_d[k * 128:(k + 1) * 128, :]) for k in range(DT)]
            wv = [load(f"wv{k}", [128, D], wv_d[k * 128:(k + 1) * 128, :]) for k in range(DT)]
            wo = [load(f"wo{k}", [128, D], wo_d[k * 128:(k + 1) * 128, :]) for k in range(DT)]
            w1 = [load(f"w1{k}", [128, DFF], w1_d[k * 128:(k + 1) * 128, :]) for k in range(DT)]
            w2g = [load(f"w2g{k}", [128, D], w2g_d[k * 128:(k + 1) * 128, :]) for k in range(FT)]
            bq = load("bq", [128, DT], bq_d[:, :], F32)
            bk = load("bk", [128, DT], bk_d[:, :], F32)
            bo = load("bo", [128, DT], bo_d[:, :], F32)
            b1 = load("b1", [128, FT], b1_d[:, :], F32)
            bvb = load("bvb", [128, DT], bvb_d[:, :], F32)
            b1l = load("b1l", [128, DT], b1l_d[:, :], F32)
            b2l = load("b2l", [128, DT], b2l_d[:, :], F32)
            bvr = load("bvr", [1, D], bvr_d[:, :])
            vgn = load("vgn", [1, D], vgn_d[:, :])
            g1p = load("g1p", [1, D], g1p_d[:, :])
            g1n = load("g1n", [1, D], g1n_d[:, :])
            g2p = load("g2p", [1, D], g2p_d[:, :])
            g2n = load("g2n", [1, D], g2n_d[:, :])
            actw = load("actw", [128, DT], actw_d[:, :])
            actb = load("actb", [128, 1], actb_d[:, :], F32)
            onesc = load("onesc", [128, 1], onesc_d[:, :])
            onesr = load("onesr", [1, 128], onesr_d[:, :])
            ehA = load("ehA", [1, 128], ehA_d[:, :])
            ehB = load("ehB", [1, 128], ehB_d[:, :])
            eye = load("eye", [128, 128], eye_d[:, :])

            st32 = [pp.tile([128, S], F32, name=f"st32_{d}") for d in range(DT)]
            st16 = [pp.tile([128, S], F16, name=f"st16_{d}") for d in range(DT)]
            kT = [pp.tile([128, S], F16, name=f"kT{d}") for d in range(DT)]
            vn = [pp.tile([128, D], F16, name=f"vn{s}") for s in range(ST)]
            qT = [pp.tile([128, CH], F16, name=f"qT{d}") for d in range(DT)]
            ctx = [pp.tile([128, CH], F16, name=f"ctx{d}") for d in range(DT)]
            o1p = [pp.tile([128, CH], F16, name=f"o1p{d}") for d in range(DT)]
            o1 = [pp.tile([128, CH], F16, name=f"o1{d}") for d in range(DT)]
            pre2 = [pp.tile([128, CH], F16, name=f"pre2_{d}") for d in range(DT)]
            new16 = [pp.tile([128, CH], F16, name=f"new16_{d}") for d in range(DT)]
            h16 = [pp.tile([128, CH], F16, name=f"h{f}") for f in range(FT)]
            hp_tm = pp.tile([128, ST], F32, name="hp_tm")      # halting prob, tile-major
            uw_row = pp.tile([1, S], F16, name="uw_row")       # update weight, seq order
            bvb_b = pp.tile([128, D], F32, name="bvb_b")       # bcast of v-bias row
            # LN stat rows (all at partition base 0)
            m_row = pp.tile([1, CH], F32, name="m_row")
            m2_row = pp.tile([1, CH], F32, name="m2_row")
            v_row = pp.tile([1, CH], F32, name="v_row")        # var -> sd -> rstd

            nc.vector.memset(hp_tm, 0.0)
            eps_t = pp.tile([1, 1], F32, name="eps_t")
            nc.vector.memset(eps_t, EPS)

            # broadcast v-bias row to all 128 partitions once
            for c2 in range(2):
                ps = ps_bc.tile([128, 256], F32, name="bv_bc", tag="bc")
                nc.tensor.matmul(ps[:, :], onesr[:, :], bvr[:, c2 * 256:(c2 + 1) * 256],
                                 start=True, stop=True)
                nc.vector.tensor_copy(bvb_b[:, c2 * 256:(c2 + 1) * 256], ps[:, :])

            # ---------------- load x, transpose into stT ----------------
            for s in range(ST):
                xs32 = pe.tile([128, D], F32, name="xs32", tag="evac")
                nc.sync.dma_start(out=xs32, in_=x_d[s * 128:(s + 1) * 128, :])
                x16 = psqv.tile([128, D], F16, name="x16", tag="sqv")
                nc.vector.tensor_copy(x16[:, :], xs32[:, :])
                for d in range(DT):
                    ps = ps_bc.tile([128, 128], F16, name="tr_in", tag="bc")
                    nc.tensor.transpose(ps[:, :], x16[:, d * 128:(d + 1) * 128], eye[:, :])
                    nc.vector.tensor_copy(st32[d][:, s * 128:(s + 1) * 128], ps[:, :])
                    nc.vector.tensor_copy(st16[d][:, s * 128:(s + 1) * 128], ps[:, :])

            # helper: LN stats for c-local fp16 tiles (sq_engine: 'act'|'dve')
            # -> fills m_row (mean) and v_row (rstd); both at base 0.
            def ln_stats(tiles, n_feat, tag, sq_act):
                ps = ps_sm.tile([64, CH], F32, name=f"st_{tag}", tag="sm")
                nt = len(tiles)
                for i, t in enumerate(tiles):
                    nc.tensor.matmul(ps[0:1, :], onesc[:, :], t[:, :],
                                     start=(i == 0), stop=(i == nt - 1))
                for i, t in enumerate(tiles):
                    if sq_act:
                        sq = psqa.tile([128, CH], F16, name=f"sqa_{tag}", tag="sqa")
                        nc.scalar.activation(out=sq[:, :], in_=t[:, :], func=AF.Square)
                    else:
                        sq = psqv.tile([128, CH], F16, name=f"sqv_{tag}", tag="sqv")
                        nc.vector.tensor_mul(sq[:, :], t[:, :], t[:, :])
                    nc.tensor.matmul(ps[32:33, :], onesc[:, :], sq[:, :],
                                     start=(i == 0), stop=(i == nt - 1))
                nc.vector.tensor_scalar(m_row, ps[0:1, :], 1.0 / n_feat, None, op0=OP.mult)
                nc.vector.tensor_mul(m2_row, m_row, m_row)
                # var = sum(x^2)/n - mean^2
                nc.vector.scalar_tensor_tensor(
                    out=v_row, in0=ps[32:33, :], scalar=1.0 / n_feat, in1=m2_row,
                    op0=OP.mult, op1=OP.subtract)
                nc.scalar.activation(out=v_row, in_=v_row, func=AF.Sqrt, bias=eps_t[:, :])
                nc.vector.reciprocal(out=v_row, in_=v_row)   # rstd

            # ---------------- the ACT steps ----------------
            for step in range(STEPS):
                # --- [A] ponder prob + halting (tile-major [128, ST]) ---
                ps_p = ps_sm.tile([128, ST], F32, name="ps_p", tag="sm")
                for s in range(ST):
                    for k in range(DT):
                        nc.tensor.matmul(
                            ps_p[:, s:s + 1], st16[k][:, s * 128:(s + 1) * 128],
                            actw[:, k:k + 1], start=(k == 0), stop=(k == DT - 1))
                p_tm = prt.tile([128, ST], F32, name="p_tm")
                nc.scalar.activation(out=p_tm, in_=ps_p[:, :], func=AF.Sigmoid,
                                     bias=actb[:, :])
                s0 = prt.tile([128, ST], F32, name="s0")
                nc.vector.tensor_scalar(s0, hp_tm, 1.0, None, op0=OP.is_lt)
                ps0 = prt.tile([128, ST], F32, name="ps0")
                nc.vector.tensor_mul(ps0, p_tm, s0)
                cand = prt.tile([128, ST], F32, name="cand")
                nc.vector.tensor_add(cand, hp_tm, ps0)
                nh = prt.tile([128, ST], F32, name="nh")
                nc.vector.tensor_scalar(nh, cand, THR, None, op0=OP.is_gt)
                nc.vector.tensor_mul(nh, nh, s0)
                s1 = prt.tile([128, ST], F32, name="s1")
                nc.vector.tensor_scalar(s1, cand, THR, None, op0=OP.is_le)
                nc.vector.tensor_mul(s1, s1, s0)
                pst = prt.tile([128, ST], F32, name="pst")
                nc.vector.tensor_mul(pst, p_tm, s1)
                nc.vector.tensor_add(hp_tm, hp_tm, pst)
                hm1 = prt.tile([128, ST], F32, name="hm1")
                nc.vector.tensor_scalar(hm1, hp_tm, 1.0, None, op0=OP.subtract)
                rem = prt.tile([128, ST], F32, name="rem")
                nc.vector.tensor_mul(rem, nh, hm1)           # -nh*(1-hp)
                nc.vector.tensor_sub(hp_tm, hp_tm, rem)      # hp += nh*(1-hp)
                uw_tm = prt.tile([128, ST], F16, name="uw_tm")
                nc.vector.tensor_sub(uw_tm, pst, rem)        # p*still + nh*(1-hp)
                # reorder tile-major -> sequence order via DRAM bounce (the
                # two DMAs share one HWDGE FIFO ring, so they stay ordered)
                nc.sync.dma_start(out=uw_scr[:, :], in_=uw_tm[:, :])
                nc.sync.dma_start(out=uw_row[0:1, :],
                                  in_=uw_scr.rearrange("p j -> j p")[:, :])

                # --- [B] kT (transposed) and v (natural) for all s ---
                for m in range(DT):
                    for c in range(NC):
                        sl = slice(c * CH, (c + 1) * CH)
                        ps = ps_mm.tile([128, CH], F32, name="kproj", tag="mm")
                        for k in range(DT):
                            nc.tensor.matmul(
                                ps[:, :], wk[k][:, m * 128:(m + 1) * 128],
                                st16[k][:, sl], start=(k == 0), stop=(k == DT - 1))
                        nc.vector.tensor_scalar_add(kT[m][:, sl], ps[:, :],
                                                    bk[:, m:m + 1])
                for s in range(ST):
                    ssl = slice(s * 128, (s + 1) * 128)
                    ps = ps_mm.tile([128, D], F32, name="vproj", tag="mm")
                    for k in range(DT):
                        nc.tensor.matmul(ps[:, :], st16[k][:, ssl], wv[k][:, :],
                                         start=(k == 0), stop=(k == DT - 1))
                    nc.vector.tensor_add(vn[s][:, :], ps[:, :], bvb_b[:, :])

                # --- per s-chunk: q-proj, attention, FFN block, st update ---
                for c in range(NC):
                    sl = slice(c * CH, (c + 1) * CH)

                    # qT for this chunk
                    for m in range(DT):
                        ps = ps_mm.tile([128, CH], F32, name="qproj", tag="mm")
                        for k in range(DT):
                            nc.tensor.matmul(
                                ps[:, :], wq[k][:, m * 128:(m + 1) * 128],
                                st16[k][:, sl], start=(k == 0), stop=(k == DT - 1))
                        nc.vector.tensor_scalar_add(qT[m][:, :], ps[:, :],
                                                    bq[:, m:m + 1])

                    # attention, per head pair
                    for t in range(H // 2):
                        at = {}
                        ps_s = ps_sm.tile([64, CH], F32, name="sums", tag="sm")
                        ps_c = ps_bc.tile([128, CH], F32, name="ctxps", tag="bc")
                        for ha in (0, 1):
                            hsl = slice(ha * 64, (ha + 1) * 64)
                            for kt in range(ST):
                                ksl = slice(kt * 128, (kt + 1) * 128)
                                psl = ps_mm.tile([128, CH], F32, name="logits", tag="mm")
                                nc.tensor.matmul(psl[:, :], kT[t][hsl, ksl],
                                                 qT[t][hsl, :], start=True, stop=True)
                                a = pa.tile([128, CH], F16, name="attnT", tag="attn")
                                nc.scalar.activation(out=a[:, :], in_=psl[:, :],
                                                     func=AF.Exp, scale=0.125)
                                at[(ha, kt)] = a
                        for ha in (0, 1):
                            for kt in range(ST):
                                nc.tensor.matmul(ps_s[32 * ha:32 * ha + 1, :],
                                                 onesc[:, :], at[(ha, kt)][:, :],
                                                 start=(kt == 0), stop=(kt == ST - 1))
                        for ha in (0, 1):
                            dsl = slice((2 * t + ha) * 64, (2 * t + ha) * 64 + 64)
                            osl = slice(ha * 64, (ha + 1) * 64)
                            for kt in range(ST):
                                nc.tensor.matmul(ps_c[osl, :], vn[kt][:, dsl],
                                                 at[(ha, kt)][:, :],
                                                 start=(kt == 0), stop=(kt == ST - 1))
                        rcpA = pcr.tile([1, CH], F16, name="rcpA", tag="crow")
                        rcpB = pcr.tile([1, CH], F16, name="rcpB", tag="crow")
                        nc.vector.reciprocal(out=rcpA, in_=ps_s[0:1, :])
                        nc.vector.reciprocal(out=rcpB, in_=ps_s[32:33, :])
                        ps_rb = ps_bc.tile([128, CH], F32, name="rbps", tag="bc")
                        nc.tensor.matmul(ps_rb[:, :], ehA[:, :], rcpA[:, :],
                                         start=True, stop=False)
                        nc.tensor.matmul(ps_rb[:, :], ehB[:, :], rcpB[:, :],
                                         start=False, stop=True)
                        rb32 = pe.tile([128, CH], F32, name="rb32", tag="evac")
                        nc.vector.tensor_copy(rb32[:, :], ps_rb[:, :])
                        nc.vector.tensor_mul(ctx[t][:, :], ps_c[:, :], rb32[:, :])

                    # --- output projection + residual (o1p) ---
                    for m in range(DT):
                        ps = ps_mm.tile([128, CH], F32, name="oproj", tag="mm")
                        for k in range(DT):
                            nc.tensor.matmul(ps[:, :], wo[k][:, m * 128:(m + 1) * 128],
                                             ctx[k][:, :], start=(k == 0),
                                             stop=(k == DT - 1))
                        nc.vector.scalar_tensor_tensor(
                            out=o1p[m][:, :], in0=ps[:, :], scalar=bo[:, m:m + 1],
                            in1=st32[m][:, sl], op0=OP.add, op1=OP.add)

                    # --- LN1 ---
                    ln_stats(o1p, D, "ln1", sq_act=False)
                    a16 = pcr.tile([1, CH], F16, name="a16", tag="crow")
                    nc.vector.tensor_copy(a16, v_row)
                    ct16 = pcr.tile([1, CH], F16, name="ct16", tag="crow")
                    nc.vector.tensor_mul(ct16, m_row, v_row)   # mean*rstd
                    for m in range(DT):
                        msl = slice(m * 128, (m + 1) * 128)
                        psA = ps_bc.tile([128, CH], F32, name="psA", tag="bc")
                        nc.tensor.matmul(psA[:, :], g1p[:, msl], a16[:, :],
                                         start=True, stop=True)
                        psC = ps_bc.tile([128, CH], F32, name="psC", tag="bc")
                        nc.tensor.matmul(psC[:, :], g1n[:, msl], ct16[:, :],
                                         start=True, stop=True)
                        tmp = psqv.tile([128, CH], F16, name="lntmp", tag="sqv")
                        nc.vector.tensor_mul(tmp[:, :], o1p[m][:, :], psA[:, :])
                        nc.vector.scalar_tensor_tensor(
                            out=o1[m][:, :], in0=tmp[:, :], scalar=b1l[:, m:m + 1],
                            in1=psC[:, :], op0=OP.add, op1=OP.add)

                    # --- FFN1 + relu ---
                    for f in range(FT):
                        ps = ps_mm.tile([128, CH], F32, name="ffn1", tag="mm")
                        for k in range(DT):
                            nc.tensor.matmul(ps[:, :], w1[k][:, f * 128:(f + 1) * 128],
                                             o1[k][:, :], start=(k == 0),
                                             stop=(k == DT - 1))
                        nc.scalar.activation(out=h16[f][:, :], in_=ps[:, :],
                                             func=AF.Relu, bias=b1[:, f:f + 1])

                    # --- lnf stats (apply folded into w2g/vgn/bvb) ---
                    ln_stats(h16, DFF, "lnf", sq_act=True)
                    mu16 = pcr.tile([1, CH], F16, name="mu16", tag="crow")
                    nc.vector.tensor_copy(mu16, m_row)
                    af16 = pcr.tile([1, CH], F16, name="af16", tag="crow")
                    nc.vector.tensor_copy(af16, v_row)
                    ps_ab = ps_bc.tile([128, CH], F32, name="ab_ps", tag="bc")
                    nc.tensor.matmul(ps_ab[:, :], onesr[:, :], af16[:, :],
                                     start=True, stop=True)
                    ab32 = pe.tile([128, CH], F32, name="ab32", tag="evac")
                    nc.vector.tensor_copy(ab32[:, :], ps_ab[:, :])

                    # --- FFN2 (lnf folded) + residual -> pre2 ---
                    for m in range(DT):
                        msl = slice(m * 128, (m + 1) * 128)
                        ps = ps_mm.tile([128, CH], F32, name="ffn2", tag="mm")
                        for k in range(FT):
                            nc.tensor.matmul(ps[:, :], w2g[k][:, msl], h16[k][:, :],
                                             start=(k == 0), stop=False)
                        nc.tensor.matmul(ps[:, :], vgn[:, msl], mu16[:, :],
                                         start=False, stop=True)
                        t32 = pe.tile([128, CH], F32, name="t32", tag="evac")
                        nc.vector.tensor_mul(t32[:, :], ps[:, :], ab32[:, :])
                        nc.vector.scalar_tensor_tensor(
                            out=pre2[m][:, :], in0=t32[:, :], scalar=bvb[:, m:m + 1],
                            in1=o1[m][:, :], op0=OP.add, op1=OP.add)

                    # --- LN2 -> new16 ---
                    ln_stats(pre2, D, "ln2", sq_act=False)
                    a216 = pcr.tile([1, CH], F16, name="a216", tag="crow")
                    nc.vector.tensor_copy(a216, v_row)
                    c216 = pcr.tile([1, CH], F16, name="c216", tag="crow")
                    nc.vector.tensor_mul(c216, m_row, v_row)
                    for m in range(DT):
                        msl = slice(m * 128, (m + 1) * 128)
                        psA = ps_bc.tile([128, CH], F32, name="psA2", tag="bc")
                        nc.tensor.matmul(psA[:, :], g2p[:, msl], a216[:, :],
                                         start=True, stop=True)
                        psC = ps_bc.tile([128, CH], F32, name="psC2", tag="bc")
                        nc.tensor.matmul(psC[:, :], g2n[:, msl], c216[:, :],
                                         start=True, stop=True)
                        tmp = psqv.tile([128, CH], F16, name="ln2tmp", tag="sqv")
                        nc.vector.tensor_mul(tmp[:, :], pre2[m][:, :], psA[:, :])
                        nc.vector.scalar_tensor_tensor(
                            out=new16[m][:, :], in0=tmp[:, :], scalar=b2l[:, m:m + 1],
                            in1=psC[:, :], op0=OP.add, op1=OP.add)

                    # --- st += uw_bcast * (new - st) ---
                    ps_uw = ps_bc.tile([128, CH], F32, name="uwps", tag="bc")
                    nc.tensor.matmul(ps_uw[:, :], onesr[:, :], uw_row[:, sl],
                                     start=True, stop=True)
                    for m in range(DT):
                        diff = pe.tile([128, CH], F32, name="diff", tag="evac")
                        nc.vector.tensor_sub(diff[:, :], new16[m][:, :], st32[m][:, sl])
                        upd = pe.tile([128, CH], F32, name="upd", tag="evac")
                        nc.vector.tensor_mul(upd[:, :], diff[:, :], ps_uw[:, :])
                        nc.vector.tensor_add(st32[m][:, sl], st32[m][:, sl], upd[:, :])
                        nc.vector.tensor_copy(st16[m][:, sl], st32[m][:, sl])

            # ---------------- transpose st back to natural, store ----------------
            for s in range(ST):
                on32 = pe.tile([128, D], F32, name="on32", tag="evac")
                for d in range(DT):
                    ps = ps_bc.tile([128, 128], F16, name="tr_out", tag="bc")
                    nc.tensor.transpose(ps[:, :], st16[d][:, s * 128:(s + 1) * 128],
                                        eye[:, :])
                    nc.vector.tensor_copy(on32[:, d * 128:(d + 1) * 128], ps[:, :])
                nc.sync.dma_start(out=out_d[s * 128:(s + 1) * 128, :], in_=on32[:, :])

    split_excess_waits(nc)
    return nc


_NC_CACHE = {}


def _get_nc():
    if "nc" not in _NC_CACHE:
        _NC_CACHE["nc"] = _build()
    return _NC_CACHE["nc"]


def kernel(**inputs):
    f16, f32 = np.float16, np.float32
    get = lambda n: np.asarray(inputs[n], f32)
    x = get("x")
    wq, bq_, wk, bk_, wv, bv_ = (get(n) for n in ("wq", "bq", "wk", "bk", "wv", "bv"))
    wo, bo_, w1, b1_ = (get(n) for n in ("wo", "bo", "w1", "b1"))
    lnf_g, lnf_b, w2, b2_ = (get(n) for n in ("lnf_g", "lnf_b", "w2", "b2"))
    ln1_g, ln1_b, ln2_g, ln2_b = (get(n) for n in ("ln1_g", "ln1_b", "ln2_g", "ln2_b"))
    act_w, act_b = get("act_w"), get("act_b")

    col = lambda v: np.ascontiguousarray(v.reshape(-1, 128).T).astype(f32)  # [128, nt]
    w2g = (w2 * lnf_g[:, None])
    vg = w2g.sum(axis=0)                     # [D]
    vb = (w2 * lnf_b[:, None]).sum(axis=0)   # [D]
    eh = np.zeros((2, 128), f16)
    eh[0, :64] = 1.0
    eh[1, 64:] = 1.0

    common = {
        "wq16": wq.astype(f16), "wk16": wk.astype(f16), "wv16": wv.astype(f16),
        "wo16": wo.astype(f16), "w116": w1.astype(f16), "w2g16": w2g.astype(f16),
        "bq32": col(bq_), "bk32": col(bk_), "bo32": col(bo_), "b132": col(b1_),
        "bvb32": col(vb + b2_), "b1ln32": col(ln1_b), "b2ln32": col(ln2_b),
        "bv16r": bv_.reshape(1, D).astype(f16),
        "vgn16": (-vg).reshape(1, D).astype(f16),
        "g1p16": ln1_g.reshape(1, D).astype(f16),
        "g1n16": (-ln1_g).reshape(1, D).astype(f16),
        "g2p16": ln2_g.reshape(1, D).astype(f16),
        "g2n16": (-ln2_g).reshape(1, D).astype(f16),
        "actw16": np.ascontiguousarray(act_w.reshape(DT, 128).T).astype(f16),
        "actb32": np.full((128, 1), float(np.ravel(act_b)[0]), f32),
        "ones_col16": np.ones((128, 1), f16),
        "ones_row16": np.ones((1, 128), f16),
        "ehA16": eh[0:1], "ehB16": eh[1:2],
        "eye16": np.eye(128, dtype=f16),
    }
    in_maps = [dict(common, x32=np.ascontiguousarray(x[i])) for i in range(N_CORES)]

    nc = _get_nc()
    res = run_bass_kernel_spmd(nc, in_maps, list(range(N_CORES)))
    return np.stack([res.results[i]["out32"] for i in range(N_CORES)]).astype(f32)


# revision 12
# speedup vs baseline: 1.0769x; 1.0769x over previous
"""ACT universal-transformer encoder (nn_Encoder_38165079392904) on 8 TRN2 cores.

Strategy: pure data-parallel over batch (B=8 -> 1 batch element per core, no
collectives). Per core, activations are kept in "transposed" layout
[feature_dim on partitions, sequence on free], so every GEMM is a natural
Trainium matmul (out = lhsT.T @ rhs, contracting over partitions):

  stT [D=512, S=1024]:  qT,kT = W.T @ stT (transposed); v = stT.T @ Wv (natural)
  logitsT[k,q] = kT_h.T @ qT_h    per head (K=dh=64, two heads share PE rows)
  attnT = exp(logitsT/8)          ACT engine, psum -> fp16
  sums  = ones.T @ attnT          matmul ones-trick (col-paired head pairs)
  ctxT  = v_h.T @ attnT           col-paired head pairs in one psum
  normalization / LN stat rows are broadcast across partitions with K=1
  matmuls; LayerNorm over dff is folded into w2 host-side (w2g = w2*lnf_g).
  ACT halting runs in fp32 on a [128, 8] tile-major layout; the update
  weight row is re-ordered to sequence order by an SBUF->SBUF DMA.

Matmul operands are fp16 (this toolchain's walrus allows only ONE semaphore
wait per ISA instruction; fp16 matmuls lower to LDWEIGHTS+MATMUL and get two
slots, while 4-byte fp32/fp32r matmuls are single self-loading instructions
and routinely fail codegen under Tile). PSUM accumulation is fp32; the
cross-step state st and halting probabilities are fp32.

A post-pass (split_excess_waits) hoists any wait beyond the per-instruction
budget into standalone EventSemaphore instructions on the same engine.
"""

import sys

sys.path.insert(0, "/opt/trn_rl_repo")

import numpy as np
import concourse.bass as bass
import concourse.tile as tile
from concourse import mybir
from concourse.bass_utils import run_bass_kernel_spmd

B, S, D, H, DFF, STEPS = 8, 1024, 512, 8, 2048, 6
DH = D // H          # 64
DT = D // 128        # 4 d-tiles
FT = DFF // 128      # 16 f-tiles
ST = S // 128        # 8 s-tiles
NC = 2               # s-chunks
CH = S // NC         # 512
EPS = 1e-6
THR = 1.0 - 0.01

F32 = mybir.dt.float32
F16 = mybir.dt.float16
AF = mybir.ActivationFunctionType
OP = mybir.AluOpType

N_CORES = 8

# parameter blob layouts (order shared by host packing and device loads)
SPEC16 = [
    ("wq", (D, D)), ("wk", (D, D)), ("wv", (D, D)), ("wo", (D, D)),
    ("w1", (D, DFF)), ("w2g", (DFF, D)),
    ("bvr", (1, D)), ("vgn", (1, D)),
    ("g1p", (1, D)), ("g1n", (1, D)), ("g2p", (1, D)), ("g2n", (1, D)),
    ("actw", (128, DT)),
    ("onesc", (128, 1)), ("onesr", (1, 128)),
    ("ehA", (1, 128)), ("ehB", (1, 128)), ("eye", (128, 128)),
]
SPEC32 = [
    ("bq", (128, DT)), ("bk", (128, DT)), ("bo", (128, DT)),
    ("b1", (128, FT)), ("bvb", (128, DT)), ("b1l", (128, DT)),
    ("b2l", (128, DT)), ("actb", (128, 1)),
]


def _offsets(spec):
    off, table = 0, {}
    for name, (r, c) in spec:
        table[name] = (off, (r, c))
        off += r * c
    return table, off


OFF16, TOT16 = _offsets(SPEC16)
OFF32, TOT32 = _offsets(SPEC32)



def split_excess_waits(nc):
    """Walrus codegen allows one sem-wait per ISA instruction (two for 2-byte
    matmuls via the LDWEIGHTS+MATMUL split). Hoist excess waits into
    standalone EventSemaphore instructions on the same engine, just before
    the instruction, preserving program order and semantics."""
    cnt = 0
    for fn in nc.m.functions:
        for bb in fn.blocks:
            il = bb.instructions
            i = 0
            while i < len(il):
                inst = il[i]
                si = inst.sync_info
                if si is not None and si.on_wait:
                    waits = list(si.on_wait)
                    cap = 1
                    if len(waits) > cap:
                        keep, extra = waits[-cap:], waits[:-cap]
                        for w in extra:
                            ni = mybir.InstEventSemaphore(
                                name=f"{inst.name}-xw{cnt}", ins=[], outs=[],
                                sync_info=mybir.SyncInfo(on_wait=[w], on_update=[]))
                            cnt += 1
                            ni.engine = inst.engine
                            il.insert(i, ni)
                            i += 1
                        inst.sync_info = mybir.SyncInfo(
                            on_wait=keep, on_update=list(si.on_update))
                i += 1
    return cnt


def _build(steps=STEPS):
    nc = bass.Bass()
    dp = lambda name, shape, dt=F16: nc.declare_dram_parameter(
        name, shape, dt, isOutput=False)

    x_d = dp("x32", [S, D], F32)
    pk16_d = dp("pk16", [TOT16], F16)
    pk32_d = dp("pk32", [TOT32], F32)
    out_d = nc.declare_dram_parameter("out32", [S, D], F32, isOutput=True)

    def src16(name, r0=0, rows=None):
        off, (R, C) = OFF16[name]
        rows = R if rows is None else rows
        return pk16_d[off + r0 * C: off + (r0 + rows) * C].rearrange(
            "(p f) -> p f", p=rows)

    def src32(name):
        off, (R, C) = OFF32[name]
        return pk32_d[off: off + R * C].rearrange("(p f) -> p f", p=R)
    uw_scr = nc.dram_tensor("uw_scratch", [128, ST], F16)

    with tile.TileContext(nc) as tc, nc.allow_low_precision(
            reason="fp16 operand pipeline by design; fp32 accumulation in PSUM"):
        with (
            tc.tile_pool(name="persist", bufs=1) as pp,
            tc.tile_pool(name="evac", bufs=6) as pe,        # f32 [128,CH] temps
            tc.tile_pool(name="sqa", bufs=6) as psqa,       # ACT-written fp16 squares
            tc.tile_pool(name="sqv", bufs=6) as psqv,       # DVE fp16 temps
            tc.tile_pool(name="attn", bufs=16) as pa,       # attnT fp16 [128,CH]
            tc.tile_pool(name="rtm", bufs=10) as prt,       # [128,8] halting temps
            tc.tile_pool(name="crow", bufs=4) as pcr,       # [1,CH] fp16 mm-rhs rows
            tc.tile_pool(name="ps_mm", bufs=4, space="PSUM") as ps_mm,
            tc.tile_pool(name="ps_bc", bufs=2, space="PSUM") as ps_bc,
            tc.tile_pool(name="ps_sm", bufs=2, space="PSUM") as ps_sm,
        ):
            # ---------------- persistent tiles + param load ----------------
            def load(name, shape, src, dt=F16):
                t = pp.tile(shape, dt, name=name)
                nc.sync.dma_start(out=t, in_=src)
                return t

            wq = [load(f"wq{k}", [128, D], src16("wq", k * 128, 128)) for k in range(DT)]
            wk = [load(f"wk{k}", [128, D], src16("wk", k * 128, 128)) for k in range(DT)]
            wv = [load(f"wv{k}", [128, D], src16("wv", k * 128, 128)) for k in range(DT)]
            wo = [load(f"wo{k}", [128, D], src16("wo", k * 128, 128)) for k in range(DT)]
            w1 = [load(f"w1{k}", [128, DFF], src16("w1", k * 128, 128)) for k in range(DT)]
            w2g = [load(f"w2g{k}", [128, D], src16("w2g", k * 128, 128)) for k in range(FT)]
            bq = load("bq", [128, DT], src32("bq"), F32)
            bk = load("bk", [128, DT], src32("bk"), F32)
            bo = load("bo", [128, DT], src32("bo"), F32)
            b1 = load("b1", [128, FT], src32("b1"), F32)
            bvb = load("bvb", [128, DT], src32("bvb"), F32)
            b1l = load("b1l", [128, DT], src32("b1l"), F32)
            b2l = load("b2l", [128, DT], src32("b2l"), F32)
            bvr = load("bvr", [1, D], src16("bvr"))
            vgn = load("vgn", [1, D], src16("vgn"))
            g1p = load("g1p", [1, D], src16("g1p"))
            g1n = load("g1n", [1, D], src16("g1n"))
            g2p = load("g2p", [1, D], src16("g2p"))
            g2n = load("g2n", [1, D], src16("g2n"))
            actw = load("actw", [128, DT], src16("actw"))
            actb = load("actb", [128, 1], src32("actb"), F32)
            onesc = load("onesc", [128, 1], src16("onesc"))
            onesr = load("onesr", [1, 128], src16("onesr"))
            ehA = load("ehA", [1, 128], src16("ehA"))
            ehB = load("ehB", [1, 128], src16("ehB"))
            eye = load("eye", [128, 128], src16("eye"))

            st32 = [pp.tile([128, S], F32, name=f"st32_{d}") for d in range(DT)]
            st16 = [pp.tile([128, S], F16, name=f"st16_{d}") for d in range(DT)]
            kT = [pp.tile([128, S], F16, name=f"kT{d}") for d in range(DT)]
            vn = [pp.tile([128, D], F16, name=f"vn{s}") for s in range(ST)]
            qT = [pp.tile([128, CH], F16, name=f"qT{d}") for d in range(DT)]
            ctx = [pp.tile([128, CH], F16, name=f"ctx{d}") for d in range(DT)]
            o1p = [pp.tile([128, CH], F16, name=f"o1p{d}") for d in range(DT)]
            o1 = [pp.tile([128, CH], F16, name=f"o1{d}") for d in range(DT)]
            pre2 = [pp.tile([128, CH], F16, name=f"pre2_{d}") for d in range(DT)]
            new16 = [pp.tile([128, CH], F16, name=f"new16_{d}") for d in range(DT)]
            h16 = [pp.tile([128, CH], F16, name=f"h{f}") for f in range(FT)]
            hp_tm = pp.tile([128, ST], F32, name="hp_tm")      # halting prob, tile-major
            uw_row = pp.tile([1, S], F16, name="uw_row")       # update weight, seq order
            bvb_b = pp.tile([128, D], F32, name="bvb_b")       # bcast of v-bias row
            # LN stat rows (all at partition base 0)
            m_row = pp.tile([1, CH], F32, name="m_row")
            m2_row = pp.tile([1, CH], F32, name="m2_row")
            v_row = pp.tile([1, CH], F32, name="v_row")        # var -> sd -> rstd

            nc.vector.memset(hp_tm, 0.0)
            eps_t = pp.tile([1, 1], F32, name="eps_t")
            nc.vector.memset(eps_t, EPS)

            # broadcast v-bias row to all 128 partitions once
            for c2 in range(2):
                ps = ps_bc.tile([128, 256], F32, name="bv_bc", tag="bc")
                nc.tensor.matmul(ps[:, :], onesr[:, :], bvr[:, c2 * 256:(c2 + 1) * 256],
                                 start=True, stop=True)
                nc.vector.tensor_copy(bvb_b[:, c2 * 256:(c2 + 1) * 256], ps[:, :])

            # ---------------- load x, transpose into stT ----------------
            for s in range(ST):
                xs32 = pe.tile([128, D], F32, name="xs32", tag="evac")
                nc.sync.dma_start(out=xs32, in_=x_d[s * 128:(s + 1) * 128, :])
                x16 = psqv.tile([128, D], F16, name="x16", tag="sqv")
                nc.vector.tensor_copy(x16[:, :], xs32[:, :])
                for d in range(DT):
                    ps = ps_bc.tile([128, 128], F16, name="tr_in", tag="bc")
                    nc.tensor.transpose(ps[:, :], x16[:, d * 128:(d + 1) * 128], eye[:, :])
                    nc.vector.tensor_copy(st32[d][:, s * 128:(s + 1) * 128], ps[:, :])
                    nc.vector.tensor_copy(st16[d][:, s * 128:(s + 1) * 128], ps[:, :])

            # helper: LN stats for c-local fp16 tiles (sq_engine: 'act'|'dve')
            # -> fills m_row (mean) and v_row (rstd); both at base 0.
            def ln_stats(tiles, n_feat, tag, sq_act):
                ps = ps_sm.tile([64, CH], F32, name=f"st_{tag}", tag="sm")
                nt = len(tiles)
                for i, t in enumerate(tiles):
                    nc.tensor.matmul(ps[0:1, :], onesc[:, :], t[:, :],
                                     start=(i == 0), stop=(i == nt - 1))
                for i, t in enumerate(tiles):
                    if sq_act:
                        sq = psqa.tile([128, CH], F16, name=f"sqa_{tag}", tag="sqa")
                        nc.scalar.activation(out=sq[:, :], in_=t[:, :], func=AF.Square)
                    else:
                        sq = psqv.tile([128, CH], F16, name=f"sqv_{tag}", tag="sqv")
                        nc.vector.tensor_mul(sq[:, :], t[:, :], t[:, :])
                    nc.tensor.matmul(ps[32:33, :], onesc[:, :], sq[:, :],
                                     start=(i == 0), stop=(i == nt - 1))
                nc.vector.tensor_scalar(m_row, ps[0:1, :], 1.0 / n_feat, None, op0=OP.mult)
                nc.vector.tensor_mul(m2_row, m_row, m_row)
                # var = sum(x^2)/n - mean^2
                nc.vector.scalar_tensor_tensor(
                    out=v_row, in0=ps[32:33, :], scalar=1.0 / n_feat, in1=m2_row,
                    op0=OP.mult, op1=OP.subtract)
                nc.scalar.activation(out=v_row, in_=v_row, func=AF.Sqrt, bias=eps_t[:, :])
                nc.vector.reciprocal(out=v_row, in_=v_row)   # rstd

            # ---------------- the ACT steps ----------------
            for step in range(steps):
                # --- [A] ponder prob + halting (tile-major [128, ST]) ---
                ps_p = ps_sm.tile([128, ST], F32, name="ps_p", tag="sm")
                for s in range(ST):
                    for k in range(DT):
                        nc.tensor.matmul(
                            ps_p[:, s:s + 1], st16[k][:, s * 128:(s + 1) * 128],
                            actw[:, k:k + 1], start=(k == 0), stop=(k == DT - 1))
                p_tm = prt.tile([128, ST], F32, name="p_tm")
                nc.scalar.activation(out=p_tm, in_=ps_p[:, :], func=AF.Sigmoid,
                                     bias=actb[:, :])
                s0 = prt.tile([128, ST], F32, name="s0")
                nc.vector.tensor_scalar(s0, hp_tm, 1.0, None, op0=OP.is_lt)
                ps0 = prt.tile([128, ST], F32, name="ps0")
                nc.vector.tensor_mul(ps0, p_tm, s0)
                cand = prt.tile([128, ST], F32, name="cand")
                nc.vector.tensor_add(cand, hp_tm, ps0)
                nh = prt.tile([128, ST], F32, name="nh")
                nc.vector.tensor_scalar(nh, cand, THR, None, op0=OP.is_gt)
                nc.vector.tensor_mul(nh, nh, s0)
                s1 = prt.tile([128, ST], F32, name="s1")
                nc.vector.tensor_scalar(s1, cand, THR, None, op0=OP.is_le)
                nc.vector.tensor_mul(s1, s1, s0)
                pst = prt.tile([128, ST], F32, name="pst")
                nc.vector.tensor_mul(pst, p_tm, s1)
                nc.vector.tensor_add(hp_tm, hp_tm, pst)
                hm1 = prt.tile([128, ST], F32, name="hm1")
                nc.vector.tensor_scalar(hm1, hp_tm, 1.0, None, op0=OP.subtract)
                rem = prt.tile([128, ST], F32, name="rem")
                nc.vector.tensor_mul(rem, nh, hm1)           # -nh*(1-hp)
                nc.vector.tensor_sub(hp_tm, hp_tm, rem)      # hp += nh*(1-hp)
                uw_tm = prt.tile([128, ST], F16, name="uw_tm")
                nc.vector.tensor_sub(uw_tm, pst, rem)        # p*still + nh*(1-hp)
                # reorder tile-major -> sequence order via DRAM bounce (the
                # two DMAs share one HWDGE FIFO ring, so they stay ordered)
                nc.sync.dma_start(out=uw_scr[:, :], in_=uw_tm[:, :])
                nc.sync.dma_start(out=uw_row[0:1, :],
                                  in_=uw_scr.rearrange("p j -> j p")[:, :])

                # --- [B] kT (transposed) and v (natural) for all s ---
                for m in range(DT):
                    for c in range(NC):
                        sl = slice(c * CH, (c + 1) * CH)
                        ps = ps_mm.tile([128, CH], F32, name="kproj", tag="mm")
                        for k in range(DT):
                            nc.tensor.matmul(
                                ps[:, :], wk[k][:, m * 128:(m + 1) * 128],
                                st16[k][:, sl], start=(k == 0), stop=(k == DT - 1))
                        nc.vector.tensor_scalar_add(kT[m][:, sl], ps[:, :],
                                                    bk[:, m:m + 1])
                for s in range(ST):
                    ssl = slice(s * 128, (s + 1) * 128)
                    ps = ps_mm.tile([128, D], F32, name="vproj", tag="mm")
                    for k in range(DT):
                        nc.tensor.matmul(ps[:, :], st16[k][:, ssl], wv[k][:, :],
                                         start=(k == 0), stop=(k == DT - 1))
                    nc.vector.tensor_add(vn[s][:, :], ps[:, :], bvb_b[:, :])

                # --- per s-chunk: q-proj, attention, FFN block, st update ---
                for c in range(NC):
                    sl = slice(c * CH, (c + 1) * CH)

                    # qT for this chunk
                    for m in range(DT):
                        ps = ps_mm.tile([128, CH], F32, name="qproj", tag="mm")
                        for k in range(DT):
                            nc.tensor.matmul(
                                ps[:, :], wq[k][:, m * 128:(m + 1) * 128],
                                st16[k][:, sl], start=(k == 0), stop=(k == DT - 1))
                        nc.vector.tensor_scalar_add(qT[m][:, :], ps[:, :],
                                                    bq[:, m:m + 1])

                    # attention, per head pair
                    for t in range(H // 2):
                        at = {}
                        ps_s = ps_sm.tile([64, CH], F32, name="sums", tag="sm")
                        ps_c = ps_bc.tile([128, CH], F32, name="ctxps", tag="bc")
                        for ha in (0, 1):
                            hsl = slice(ha * 64, (ha + 1) * 64)
                            for kt in range(ST):
                                ksl = slice(kt * 128, (kt + 1) * 128)
                                psl = ps_mm.tile([128, CH], F32, name="logits", tag="mm")
                                nc.tensor.matmul(psl[:, :], kT[t][hsl, ksl],
                                                 qT[t][hsl, :], start=True, stop=True)
                                a = pa.tile([128, CH], F16, name="attnT", tag="attn")
                                nc.scalar.activation(out=a[:, :], in_=psl[:, :],
                                                     func=AF.Exp, scale=0.125)
                                at[(ha, kt)] = a
                        for ha in (0, 1):
                            for kt in range(ST):
                                nc.tensor.matmul(ps_s[32 * ha:32 * ha + 1, :],
                                                 onesc[:, :], at[(ha, kt)][:, :],
                                                 start=(kt == 0), stop=(kt == ST - 1))
                        for ha in (0, 1):
                            dsl = slice((2 * t + ha) * 64, (2 * t + ha) * 64 + 64)
                            osl = slice(ha * 64, (ha + 1) * 64)
                            for kt in range(ST):
                                nc.tensor.matmul(ps_c[osl, :], vn[kt][:, dsl],
                                                 at[(ha, kt)][:, :],
                                                 start=(kt == 0), stop=(kt == ST - 1))
                        rcpA = pcr.tile([1, CH], F16, name="rcpA", tag="crow")
                        rcpB = pcr.tile([1, CH], F16, name="rcpB", tag="crow")
                        nc.vector.reciprocal(out=rcpA, in_=ps_s[0:1, :])
                        nc.vector.reciprocal(out=rcpB, in_=ps_s[32:33, :])
                        ps_rb = ps_bc.tile([128, CH], F32, name="rbps", tag="bc")
                        nc.tensor.matmul(ps_rb[:, :], ehA[:, :], rcpA[:, :],
                                         start=True, stop=False)
                        nc.tensor.matmul(ps_rb[:, :], ehB[:, :], rcpB[:, :],
                                         start=False, stop=True)
                        rb32 = pe.tile([128, CH], F32, name="rb32", tag="evac")
                        nc.vector.tensor_copy(rb32[:, :], ps_rb[:, :])
                        nc.vector.tensor_mul(ctx[t][:, :], ps_c[:, :], rb32[:, :])

                    # --- output projection + residual (o1p) ---
                    for m in range(DT):
                        ps = ps_mm.tile([128, CH], F32, name="oproj", tag="mm")
                        for k in range(DT):
                            nc.tensor.matmul(ps[:, :], wo[k][:, m * 128:(m + 1) * 128],
                                             ctx[k][:, :], start=(k == 0),
                                             stop=(k == DT - 1))
                        nc.vector.scalar_tensor_tensor(
                            out=o1p[m][:, :], in0=ps[:, :], scalar=bo[:, m:m + 1],
                            in1=st32[m][:, sl], op0=OP.add, op1=OP.add)

                    # --- LN1 ---
                    ln_stats(o1p, D, "ln1", sq_act=False)
                    a16 = pcr.tile([1, CH], F16, name="a16", tag="crow")
                    nc.vector.tensor_copy(a16, v_row)
                    ct16 = pcr.tile([1, CH], F16, name="ct16", tag="crow")
                    nc.vector.tensor_mul(ct16, m_row, v_row)   # mean*rstd
                    for m in range(DT):
                        msl = slice(m * 128, (m + 1) * 128)
                        psA = ps_bc.tile([128, CH], F32, name="psA", tag="bc")
                        nc.tensor.matmul(psA[:, :], g1p[:, msl], a16[:, :],
                                         start=True, stop=True)
                        psC = ps_bc.tile([128, CH], F32, name="psC", tag="bc")
                        nc.tensor.matmul(psC[:, :], g1n[:, msl], ct16[:, :],
                                         start=True, stop=True)
                        tmp = psqv.tile([128, CH], F16, name="lntmp", tag="sqv")
                        nc.vector.tensor_mul(tmp[:, :], o1p[m][:, :], psA[:, :])
                        nc.vector.scalar_tensor_tensor(
                            out=o1[m][:, :], in0=tmp[:, :], scalar=b1l[:, m:m + 1],
                            in1=psC[:, :], op0=OP.add, op1=OP.add)

                    # --- FFN1 + relu ---
                    for f in range(FT):
                        ps = ps_mm.tile([128, CH], F32, name="ffn1", tag="mm")
                        for k in range(DT):
                            nc.tensor.matmul(ps[:, :], w1[k][:, f * 128:(f + 1) * 128],
                                             o1[k][:, :], start=(k == 0),
                                             stop=(k == DT - 1))
                        nc.scalar.activation(out=h16[f][:, :], in_=ps[:, :],
                                             func=AF.Relu, bias=b1[:, f:f + 1])

                    # --- lnf stats (apply folded into w2g/vgn/bvb) ---
                    ln_stats(h16, DFF, "lnf", sq_act=True)
                    mu16 = pcr.tile([1, CH], F16, name="mu16", tag="crow")
                    nc.vector.tensor_copy(mu16, m_row)
                    af16 = pcr.tile([1, CH], F16, name="af16", tag="crow")
                    nc.vector.tensor_copy(af16, v_row)
                    ps_ab = ps_bc.tile([128, CH], F32, name="ab_ps", tag="bc")
                    nc.tensor.matmul(ps_ab[:, :], onesr[:, :], af16[:, :],
                                     start=True, stop=True)
                    ab32 = pe.tile([128, CH], F32, name="ab32", tag="evac")
                    nc.vector.tensor_copy(ab32[:, :], ps_ab[:, :])

                    # --- FFN2 (lnf folded) + residual -> pre2 ---
                    for m in range(DT):
                        msl = slice(m * 128, (m + 1) * 128)
                        ps = ps_mm.tile([128, CH], F32, name="ffn2", tag="mm")
                        for k in range(FT):
                            nc.tensor.matmul(ps[:, :], w2g[k][:, msl], h16[k][:, :],
                                             start=(k == 0), stop=False)
                        nc.tensor.matmul(ps[:, :], vgn[:, msl], mu16[:, :],
                                         start=False, stop=True)
                        t32 = pe.tile([128, CH], F32, name="t32", tag="evac")
                        nc.vector.tensor_mul(t32[:, :], ps[:, :], ab32[:, :])
                        nc.vector.scalar_tensor_tensor(
                            out=pre2[m][:, :], in0=t32[:, :], scalar=bvb[:, m:m + 1],
                            in1=o1[m][:, :], op0=OP.add, op1=OP.add)

                    # --- LN2 -> new16 ---
                    ln_stats(pre2, D, "ln2", sq_act=False)
                    a216 = pcr.tile([1, CH], F16, name="a216", tag="crow")
                    nc.vector.tensor_copy(a216, v_row)
                    c216 = pcr.tile([1, CH], F16, name="c216", tag="crow")
                    nc.vector.tensor_mul(c216, m_row, v_row)
                    for m in range(DT):
                        msl = slice(m * 128, (m + 1) * 128)
                        psA = ps_bc.tile([128, CH], F32, name="psA2", tag="bc")
                        nc.tensor.matmul(psA[:, :], g2p[:, msl], a216[:, :],
                                         start=True, stop=True)
                        psC = ps_bc.tile([128, CH], F32, name="psC2", tag="bc")
                        nc.tensor.matmul(psC[:, :], g2n[:, msl], c216[:, :],
                                         start=True, stop=True)
                        tmp = psqv.tile([128, CH], F16, name="ln2tmp", tag="sqv")
                        nc.vector.tensor_mul(tmp[:, :], pre2[m][:, :], psA[:, :])
                        nc.vector.scalar_tensor_tensor(
                            out=new16[m][:, :], in0=tmp[:, :], scalar=b2l[:, m:m + 1],
                            in1=psC[:, :], op0=OP.add, op1=OP.add)

                    # --- st += uw_bcast * (new - st) ---
                    ps_uw = ps_bc.tile([128, CH], F32, name="uwps", tag="bc")
                    nc.tensor.matmul(ps_uw[:, :], onesr[:, :], uw_row[:, sl],
                                     start=True, stop=True)
                    for m in range(DT):
                        diff = pe.tile([128, CH], F32, name="diff", tag="evac")
                        nc.vector.tensor_sub(diff[:, :], new16[m][:, :], st32[m][:, sl])
                        upd = pe.tile([128, CH], F32, name="upd", tag="evac")
                        nc.vector.tensor_mul(upd[:, :], diff[:, :], ps_uw[:, :])
                        nc.vector.tensor_add(st32[m][:, sl], st32[m][:, sl], upd[:, :])
                        nc.vector.tensor_copy(st16[m][:, sl], st32[m][:, sl])

            # ---------------- transpose st back to natural, store ----------------
            for s in range(ST):
                on32 = pe.tile([128, D], F32, name="on32", tag="evac")
                for d in range(DT):
                    ps = ps_bc.tile([128, 128], F16, name="tr_out", tag="bc")
                    nc.tensor.transpose(ps[:, :], st16[d][:, s * 128:(s + 1) * 128],
                                        eye[:, :])
                    nc.vector.tensor_copy(on32[:, d * 128:(d + 1) * 128], ps[:, :])
                nc.sync.dma_start(out=out_d[s * 128:(s + 1) * 128, :], in_=on32[:, :])

    split_excess_waits(nc)
    return nc


_NC_CACHE = {}


def _get_nc():
    if "nc" not in _NC_CACHE:
        _NC_CACHE["nc"] = _build()
    return _NC_CACHE["nc"]


def kernel(**inputs):
    f16, f32 = np.float16, np.float32
    get = lambda n: np.asarray(inputs[n], f32)
    x = get("x")
    wq, bq_, wk, bk_, wv, bv_ = (get(n) for n in ("wq", "bq", "wk", "bk", "wv", "bv"))
    wo, bo_, w1, b1_ = (get(n) for n in ("wo", "bo", "w1", "b1"))
    lnf_g, lnf_b, w2, b2_ = (get(n) for n in ("lnf_g", "lnf_b", "w2", "b2"))
    ln1_g, ln1_b, ln2_g, ln2_b = (get(n) for n in ("ln1_g", "ln1_b", "ln2_g", "ln2_b"))
    act_w, act_b = get("act_w"), get("act_b")

    col = lambda v: np.ascontiguousarray(v.reshape(-1, 128).T).astype(f32)  # [128, nt]
    w2g = (w2 * lnf_g[:, None])
    vg = w2g.sum(axis=0)                     # [D]
    vb = (w2 * lnf_b[:, None]).sum(axis=0)   # [D]
    eh = np.zeros((2, 128), f16)
    eh[0, :64] = 1.0
    eh[1, 64:] = 1.0

    parts16 = {
        "wq": wq.astype(f16), "wk": wk.astype(f16), "wv": wv.astype(f16),
        "wo": wo.astype(f16), "w1": w1.astype(f16), "w2g": w2g.astype(f16),
        "bvr": bv_.reshape(1, D).astype(f16),
        "vgn": (-vg).reshape(1, D).astype(f16),
        "g1p": ln1_g.reshape(1, D).astype(f16),
        "g1n": (-ln1_g).reshape(1, D).astype(f16),
        "g2p": ln2_g.reshape(1, D).astype(f16),
        "g2n": (-ln2_g).reshape(1, D).astype(f16),
        "actw": np.ascontiguousarray(act_w.reshape(DT, 128).T).astype(f16),
        "onesc": np.ones((128, 1), f16),
        "onesr": np.ones((1, 128), f16),
        "ehA": eh[0:1], "ehB": eh[1:2],
        "eye": np.eye(128, dtype=f16),
    }
    parts32 = {
        "bq": col(bq_), "bk": col(bk_), "bo": col(bo_), "b1": col(b1_),
        "bvb": col(vb + b2_), "b1l": col(ln1_b), "b2l": col(ln2_b),
        "actb": np.full((128, 1), float(np.ravel(act_b)[0]), f32),
    }
    for name, shp in SPEC16:
        assert parts16[name].shape == shp, (name, parts16[name].shape, shp)
    for name, shp in SPEC32:
        assert parts32[name].shape == shp, (name, parts32[name].shape, shp)
    pk16 = np.concatenate([np.ascontiguousarray(parts16[n]).ravel() for n, _ in SPEC16])
    pk32 = np.concatenate([np.ascontiguousarray(parts32[n]).ravel() for n, _ in SPEC32])
    common = {"pk16": pk16, "pk32": pk32}
    in_maps = [dict(common, x32=np.ascontiguousarray(x[i])) for i in range(N_CORES)]

    nc = _get_nc()
    res = run_bass_kernel_spmd(nc, in_maps, list(range(N_CORES)))
    return np.stack([res.results[i]["out32"] for i in range(N_CORES)]).astype(f32)


# revision 13
# speedup vs baseline: 72.2574x; 67.0964x over previous
"""ACT universal-transformer encoder (nn_Encoder_38165079392904) on 8 TRN2 cores.

Strategy: pure data-parallel over batch (B=8 -> 1 batch element per core, no
collectives). Per core, activations are kept in "transposed" layout
[feature_dim on partitions, sequence on free], so every GEMM is a natural
Trainium matmul (out = lhsT.T @ rhs, contracting over partitions):

  stT [D=512, S=1024]:  qT,kT = W.T @ stT (transposed); v = stT.T @ Wv (natural)
  logitsT[k,q] = kT_h.T @ qT_h    per head (K=dh=64, two heads share PE rows)
  attnT = exp(logitsT/8)          ACT engine, psum -> fp16
  sums  = ones.T @ attnT          matmul ones-trick (col-paired head pairs)
  ctxT  = v_h.T @ attnT           col-paired head pairs in one psum
  normalization / LN stat rows are broadcast across partitions with K=1
  matmuls; LayerNorm over dff is folded into w2 host-side (w2g = w2*lnf_g).
  ACT halting runs in fp32 on a [128, 8] tile-major layout; the update
  weight row is re-ordered to sequence order by an SBUF->SBUF DMA.

Matmul operands are fp16 (this toolchain's walrus allows only ONE semaphore
wait per ISA instruction; fp16 matmuls lower to LDWEIGHTS+MATMUL and get two
slots, while 4-byte fp32/fp32r matmuls are single self-loading instructions
and routinely fail codegen under Tile). PSUM accumulation is fp32; the
cross-step state st and halting probabilities are fp32.

A post-pass (split_excess_waits) hoists any wait beyond the per-instruction
budget into standalone EventSemaphore instructions on the same engine.
"""

import sys

sys.path.insert(0, "/opt/trn_rl_repo")

import numpy as np
import concourse.bass as bass
import concourse.tile as tile
from concourse import mybir
from concourse.bass_utils import run_bass_kernel_spmd

B, S, D, H, DFF, STEPS = 8, 1024, 512, 8, 2048, 6
DH = D // H          # 64
DT = D // 128        # 4 d-tiles
FT = DFF // 128      # 16 f-tiles
ST = S // 128        # 8 s-tiles
NC = 2               # s-chunks
CH = S // NC         # 512
EPS = 1e-6
THR = 1.0 - 0.01

F32 = mybir.dt.float32
F16 = mybir.dt.float16
AF = mybir.ActivationFunctionType
OP = mybir.AluOpType

N_CORES = 8

# parameter blob layouts (order shared by host packing and device loads)
SPEC16 = [
    ("wq", (D, D)), ("wk", (D, D)), ("wv", (D, D)), ("wo", (D, D)),
    ("w1", (D, DFF)), ("w2g", (DFF, D)),
    ("bvr", (1, D)), ("vgn", (1, D)),
    ("g1p", (1, D)), ("g1n", (1, D)), ("g2p", (1, D)), ("g2n", (1, D)),
    ("actw", (128, DT)),
    ("onesc", (128, 1)), ("onesr", (1, 128)),
    ("ehA", (1, 128)), ("ehB", (1, 128)), ("eye", (128, 128)),
]
SPEC32 = [
    ("bq", (128, DT)), ("bk", (128, DT)), ("bo", (128, DT)),
    ("b1", (128, FT)), ("bvb", (128, DT)), ("b1l", (128, DT)),
    ("b2l", (128, DT)), ("actb", (128, 1)),
]


def _offsets(spec):
    off, table = 0, {}
    for name, (r, c) in spec:
        table[name] = (off, (r, c))
        off += r * c
    return table, off


OFF16, TOT16 = _offsets(SPEC16)
OFF32, TOT32 = _offsets(SPEC32)



def split_excess_waits(nc):
    """Walrus codegen allows one sem-wait per ISA instruction (two for 2-byte
    matmuls via the LDWEIGHTS+MATMUL split). Hoist excess waits into
    standalone EventSemaphore instructions on the same engine, just before
    the instruction, preserving program order and semantics."""
    cnt = 0
    for fn in nc.m.functions:
        for bb in fn.blocks:
            il = bb.instructions
            i = 0
            while i < len(il):
                inst = il[i]
                si = inst.sync_info
                if si is not None and si.on_wait:
                    waits = list(si.on_wait)
                    cap = 1
                    if len(waits) > cap:
                        keep, extra = waits[-cap:], waits[:-cap]
                        for w in extra:
                            ni = mybir.InstEventSemaphore(
                                name=f"{inst.name}-xw{cnt}", ins=[], outs=[],
                                sync_info=mybir.SyncInfo(on_wait=[w], on_update=[]))
                            cnt += 1
                            ni.engine = inst.engine
                            il.insert(i, ni)
                            i += 1
                        inst.sync_info = mybir.SyncInfo(
                            on_wait=keep, on_update=list(si.on_update))
                i += 1
    return cnt


def _build(steps=STEPS):
    nc = bass.Bass()
    dp = lambda name, shape, dt=F16: nc.declare_dram_parameter(
        name, shape, dt, isOutput=False)

    x_d = dp("x32", [S, D], F32)
    pk16_d = dp("pk16", [TOT16], F16)
    pk32_d = dp("pk32", [TOT32], F32)
    out_d = nc.declare_dram_parameter("out32", [S, D], F32, isOutput=True)

    def src16(name, r0=0, rows=None):
        off, (R, C) = OFF16[name]
        rows = R if rows is None else rows
        return pk16_d[off + r0 * C: off + (r0 + rows) * C].rearrange(
            "(p f) -> p f", p=rows)

    def src32(name):
        off, (R, C) = OFF32[name]
        return pk32_d[off: off + R * C].rearrange("(p f) -> p f", p=R)
    uw_scr = nc.dram_tensor("uw_scratch", [128, ST], F16)

    with tile.TileContext(nc) as tc, nc.allow_low_precision(
            reason="fp16 operand pipeline by design; fp32 accumulation in PSUM"):
        with (
            tc.tile_pool(name="persist", bufs=1) as pp,
            tc.tile_pool(name="evac", bufs=6) as pe,        # f32 [128,CH] temps
            tc.tile_pool(name="sqa", bufs=6) as psqa,       # ACT-written fp16 squares
            tc.tile_pool(name="sqv", bufs=6) as psqv,       # DVE fp16 temps
            tc.tile_pool(name="attn", bufs=16) as pa,       # attnT fp16 [128,CH]
            tc.tile_pool(name="rtm", bufs=10) as prt,       # [128,8] halting temps
            tc.tile_pool(name="crow", bufs=4) as pcr,       # [1,CH] fp16 mm-rhs rows
            tc.tile_pool(name="ps_mm", bufs=4, space="PSUM") as ps_mm,
            tc.tile_pool(name="ps_bc", bufs=2, space="PSUM") as ps_bc,
            tc.tile_pool(name="ps_sm", bufs=2, space="PSUM") as ps_sm,
        ):
            # ---------------- persistent tiles + param load ----------------
            def load(name, shape, src, dt=F16):
                t = pp.tile(shape, dt, name=name)
                nc.sync.dma_start(out=t, in_=src)
                return t

            wq = [load(f"wq{k}", [128, D], src16("wq", k * 128, 128)) for k in range(DT)]
            wk = [load(f"wk{k}", [128, D], src16("wk", k * 128, 128)) for k in range(DT)]
            wv = [load(f"wv{k}", [128, D], src16("wv", k * 128, 128)) for k in range(DT)]
            wo = [load(f"wo{k}", [128, D], src16("wo", k * 128, 128)) for k in range(DT)]
            w1 = [load(f"w1{k}", [128, DFF], src16("w1", k * 128, 128)) for k in range(DT)]
            w2g = [load(f"w2g{k}", [128, D], src16("w2g", k * 128, 128)) for k in range(FT)]
            bq = load("bq", [128, DT], src32("bq"), F32)
            bk = load("bk", [128, DT], src32("bk"), F32)
            bo = load("bo", [128, DT], src32("bo"), F32)
            b1 = load("b1", [128, FT], src32("b1"), F32)
            bvb = load("bvb", [128, DT], src32("bvb"), F32)
            b1l = load("b1l", [128, DT], src32("b1l"), F32)
            b2l = load("b2l", [128, DT], src32("b2l"), F32)
            bvr = load("bvr", [1, D], src16("bvr"))
            vgn = load("vgn", [1, D], src16("vgn"))
            g1p = load("g1p", [1, D], src16("g1p"))
            g1n = load("g1n", [1, D], src16("g1n"))
            g2p = load("g2p", [1, D], src16("g2p"))
            g2n = load("g2n", [1, D], src16("g2n"))
            actw = load("actw", [128, DT], src16("actw"))
            actb = load("actb", [128, 1], src32("actb"), F32)
            onesc = load("onesc", [128, 1], src16("onesc"))
            onesr = load("onesr", [1, 128], src16("onesr"))
            ehA = load("ehA", [1, 128], src16("ehA"))
            ehB = load("ehB", [1, 128], src16("ehB"))
            eye = load("eye", [128, 128], src16("eye"))

            st32 = [pp.tile([128, S], F32, name=f"st32_{d}") for d in range(DT)]
            st16 = [pp.tile([128, S], F16, name=f"st16_{d}") for d in range(DT)]
            kT = [pp.tile([128, S], F16, name=f"kT{d}") for d in range(DT)]
            vn = [pp.tile([128, D], F16, name=f"vn{s}") for s in range(ST)]
            qT = [pp.tile([128, CH], F16, name=f"qT{d}") for d in range(DT)]
            ctx = [pp.tile([128, CH], F16, name=f"ctx{d}") for d in range(DT)]
            o1p = [pp.tile([128, CH], F16, name=f"o1p{d}") for d in range(DT)]
            o1 = [pp.tile([128, CH], F16, name=f"o1{d}") for d in range(DT)]
            pre2 = [pp.tile([128, CH], F16, name=f"pre2_{d}") for d in range(DT)]
            new16 = [pp.tile([128, CH], F16, name=f"new16_{d}") for d in range(DT)]
            h16 = [pp.tile([128, CH], F16, name=f"h{f}") for f in range(FT)]
            hp_tm = pp.tile([128, ST], F32, name="hp_tm")      # halting prob, tile-major
            uw_row = pp.tile([1, S], F16, name="uw_row")       # update weight, seq order
            bvb_b = pp.tile([128, D], F32, name="bvb_b")       # bcast of v-bias row
            # LN stat rows (all at partition base 0)
            m_row = pp.tile([1, CH], F32, name="m_row")
            m2_row = pp.tile([1, CH], F32, name="m2_row")
            v_row = pp.tile([1, CH], F32, name="v_row")        # var -> sd -> rstd

            nc.vector.memset(hp_tm, 0.0)
            eps_t = pp.tile([1, 1], F32, name="eps_t")
            nc.vector.memset(eps_t, EPS)

            # broadcast v-bias row to all 128 partitions once
            for c2 in range(2):
                ps = ps_bc.tile([128, 256], F32, name="bv_bc", tag="bc")
                nc.tensor.matmul(ps[:, :], onesr[:, :], bvr[:, c2 * 256:(c2 + 1) * 256],
                                 start=True, stop=True)
                nc.vector.tensor_copy(bvb_b[:, c2 * 256:(c2 + 1) * 256], ps[:, :])

            # ---------------- load x, transpose into stT ----------------
            for s in range(ST):
                xs32 = pe.tile([128, D], F32, name="xs32", tag="evac")
                nc.sync.dma_start(out=xs32, in_=x_d[s * 128:(s + 1) * 128, :])
                x16 = psqv.tile([128, D], F16, name="x16", tag="sqv")
                nc.vector.tensor_copy(x16[:, :], xs32[:, :])
                for d in range(DT):
                    ps = ps_bc.tile([128, 128], F16, name="tr_in", tag="bc")
                    nc.tensor.transpose(ps[:, :], x16[:, d * 128:(d + 1) * 128], eye[:, :])
                    nc.vector.tensor_copy(st32[d][:, s * 128:(s + 1) * 128], ps[:, :])
                    nc.vector.tensor_copy(st16[d][:, s * 128:(s + 1) * 128], ps[:, :])

            # helper: LN stats for c-local fp16 tiles (sq_engine: 'act'|'dve')
            # -> fills m_row (mean) and v_row (rstd); both at base 0.
            def ln_stats(tiles, n_feat, tag, sq_act):
                ps = ps_sm.tile([64, CH], F32, name=f"st_{tag}", tag="sm")
                nt = len(tiles)
                for i, t in enumerate(tiles):
                    nc.tensor.matmul(ps[0:1, :], onesc[:, :], t[:, :],
                                     start=(i == 0), stop=(i == nt - 1))
                for i, t in enumerate(tiles):
                    if sq_act:
                        sq = psqa.tile([128, CH], F16, name=f"sqa_{tag}", tag="sqa")
                        nc.scalar.activation(out=sq[:, :], in_=t[:, :], func=AF.Square)
                    else:
                        sq = psqv.tile([128, CH], F16, name=f"sqv_{tag}", tag="sqv")
                        nc.vector.tensor_mul(sq[:, :], t[:, :], t[:, :])
                    nc.tensor.matmul(ps[32:33, :], onesc[:, :], sq[:, :],
                                     start=(i == 0), stop=(i == nt - 1))
                nc.vector.tensor_scalar(m_row, ps[0:1, :], 1.0 / n_feat, None, op0=OP.mult)
                nc.vector.tensor_mul(m2_row, m_row, m_row)
                # var = sum(x^2)/n - mean^2
                nc.vector.scalar_tensor_tensor(
                    out=v_row, in0=ps[32:33, :], scalar=1.0 / n_feat, in1=m2_row,
                    op0=OP.mult, op1=OP.subtract)
                nc.scalar.activation(out=v_row, in_=v_row, func=AF.Sqrt, bias=eps_t[:, :])
                nc.vector.reciprocal(out=v_row, in_=v_row)   # rstd

            # ---------------- the ACT steps ----------------
            for step in range(steps):
                # --- [A] ponder prob + halting (tile-major [128, ST]) ---
                ps_p = ps_sm.tile([128, ST], F32, name="ps_p", tag="sm")
                for s in range(ST):
                    for k in range(DT):
                        nc.tensor.matmul(
                            ps_p[:, s:s + 1], st16[k][:, s * 128:(s + 1) * 128],
                            actw[:, k:k + 1], start=(k == 0), stop=(k == DT - 1))
                p_tm = prt.tile([128, ST], F32, name="p_tm")
                nc.scalar.activation(out=p_tm, in_=ps_p[:, :], func=AF.Sigmoid,
                                     bias=actb[:, :])
                s0 = prt.tile([128, ST], F32, name="s0")
                nc.vector.tensor_scalar(s0, hp_tm, 1.0, None, op0=OP.is_lt)
                ps0 = prt.tile([128, ST], F32, name="ps0")
                nc.vector.tensor_mul(ps0, p_tm, s0)
                cand = prt.tile([128, ST], F32, name="cand")
                nc.vector.tensor_add(cand, hp_tm, ps0)
                nh = prt.tile([128, ST], F32, name="nh")
                nc.vector.tensor_scalar(nh, cand, THR, None, op0=OP.is_gt)
                nc.vector.tensor_mul(nh, nh, s0)
                s1 = prt.tile([128, ST], F32, name="s1")
                nc.vector.tensor_scalar(s1, cand, THR, None, op0=OP.is_le)
                nc.vector.tensor_mul(s1, s1, s0)
                pst = prt.tile([128, ST], F32, name="pst")
                nc.vector.tensor_mul(pst, p_tm, s1)
                nc.vector.tensor_add(hp_tm, hp_tm, pst)
                hm1 = prt.tile([128, ST], F32, name="hm1")
                nc.vector.tensor_scalar(hm1, hp_tm, 1.0, None, op0=OP.subtract)
                rem = prt.tile([128, ST], F32, name="rem")
                nc.vector.tensor_mul(rem, nh, hm1)           # -nh*(1-hp)
                nc.vector.tensor_sub(hp_tm, hp_tm, rem)      # hp += nh*(1-hp)
                uw_tm = prt.tile([128, ST], F16, name="uw_tm")
                nc.vector.tensor_sub(uw_tm, pst, rem)        # p*still + nh*(1-hp)
                # reorder tile-major -> sequence order via DRAM bounce (the
                # two DMAs share one HWDGE FIFO ring, so they stay ordered)
                nc.sync.dma_start(out=uw_scr[:, :], in_=uw_tm[:, :])
                nc.sync.dma_start(out=uw_row[0:1, :],
                                  in_=uw_scr.rearrange("p j -> j p")[:, :])

                # --- [B] kT (transposed) and v (natural) for all s ---
                for m in range(DT):
                    for c in range(NC):
                        sl = slice(c * CH, (c + 1) * CH)
                        ps = ps_mm.tile([128, CH], F32, name="kproj", tag="mm")
                        for k in range(DT):
                            nc.tensor.matmul(
                                ps[:, :], wk[k][:, m * 128:(m + 1) * 128],
                                st16[k][:, sl], start=(k == 0), stop=(k == DT - 1))
                        nc.vector.tensor_scalar_add(kT[m][:, sl], ps[:, :],
                                                    bk[:, m:m + 1])
                for s in range(ST):
                    ssl = slice(s * 128, (s + 1) * 128)
                    ps = ps_mm.tile([128, D], F32, name="vproj", tag="mm")
                    for k in range(DT):
                        nc.tensor.matmul(ps[:, :], st16[k][:, ssl], wv[k][:, :],
                                         start=(k == 0), stop=(k == DT - 1))
                    nc.vector.tensor_add(vn[s][:, :], ps[:, :], bvb_b[:, :])

                # --- per s-chunk: q-proj, attention, FFN block, st update ---
                for c in range(NC):
                    sl = slice(c * CH, (c + 1) * CH)

                    # qT for this chunk
                    for m in range(DT):
                        ps = ps_mm.tile([128, CH], F32, name="qproj", tag="mm")
                        for k in range(DT):
                            nc.tensor.matmul(
                                ps[:, :], wq[k][:, m * 128:(m + 1) * 128],
                                st16[k][:, sl], start=(k == 0), stop=(k == DT - 1))
                        nc.vector.tensor_scalar_add(qT[m][:, :], ps[:, :],
                                                    bq[:, m:m + 1])

                    # attention, per head pair
                    for t in range(H // 2):
                        at = {}
                        ps_s = ps_sm.tile([64, CH], F32, name="sums", tag="sm")
                        ps_c = ps_bc.tile([128, CH], F32, name="ctxps", tag="bc")
                        for ha in (0, 1):
                            hsl = slice(ha * 64, (ha + 1) * 64)
                            for kt in range(ST):
                                ksl = slice(kt * 128, (kt + 1) * 128)
                                psl = ps_mm.tile([128, CH], F32, name="logits", tag="mm")
                                nc.tensor.matmul(psl[:, :], kT[t][hsl, ksl],
                                                 qT[t][hsl, :], start=True, stop=True)
                                a = pa.tile([128, CH], F16, name="attnT", tag="attn")
                                nc.scalar.activation(out=a[:, :], in_=psl[:, :],
                                                     func=AF.Exp, scale=0.125)
                                at[(ha, kt)] = a
                        for ha in (0, 1):
                            for kt in range(ST):
                                nc.tensor.matmul(ps_s[32 * ha:32 * ha + 1, :],
                                                 onesc[:, :], at[(ha, kt)][:, :],
                                                 start=(kt == 0), stop=(kt == ST - 1))
                        for ha in (0, 1):
                            dsl = slice((2 * t + ha) * 64, (2 * t + ha) * 64 + 64)
                            osl = slice(ha * 64, (ha + 1) * 64)
                            for kt in range(ST):
                                nc.tensor.matmul(ps_c[osl, :], vn[kt][:, dsl],
                                                 at[(ha, kt)][:, :],
                                                 start=(kt == 0), stop=(kt == ST - 1))
                        rcpA = pcr.tile([1, CH], F16, name="rcpA", tag="crow")
                        rcpB = pcr.tile([1, CH], F16, name="rcpB", tag="crow")
                        nc.vector.reciprocal(out=rcpA, in_=ps_s[0:1, :])
                        nc.vector.reciprocal(out=rcpB, in_=ps_s[32:33, :])
                        ps_rb = ps_bc.tile([128, CH], F32, name="rbps", tag="bc")
                        nc.tensor.matmul(ps_rb[:, :], ehA[:, :], rcpA[:, :],
                                         start=True, stop=False)
                        nc.tensor.matmul(ps_rb[:, :], ehB[:, :], rcpB[:, :],
                                         start=False, stop=True)
                        rb32 = pe.tile([128, CH], F32, name="rb32", tag="evac")
                        nc.vector.tensor_copy(rb32[:, :], ps_rb[:, :])
                        nc.vector.tensor_mul(ctx[t][:, :], ps_c[:, :], rb32[:, :])

                    # --- output projection + residual (o1p) ---
                    for m in range(DT):
                        ps = ps_mm.tile([128, CH], F32, name="oproj", tag="mm")
                        for k in range(DT):
                            nc.tensor.matmul(ps[:, :], wo[k][:, m * 128:(m + 1) * 128],
                                             ctx[k][:, :], start=(k == 0),
                                             stop=(k == DT - 1))
                        nc.vector.scalar_tensor_tensor(
                            out=o1p[m][:, :], in0=ps[:, :], scalar=bo[:, m:m + 1],
                            in1=st32[m][:, sl], op0=OP.add, op1=OP.add)

                    # --- LN1 ---
                    ln_stats(o1p, D, "ln1", sq_act=False)
                    a16 = pcr.tile([1, CH], F16, name="a16", tag="crow")
                    nc.vector.tensor_copy(a16, v_row)
                    ct16 = pcr.tile([1, CH], F16, name="ct16", tag="crow")
                    nc.vector.tensor_mul(ct16, m_row, v_row)   # mean*rstd
                    for m in range(DT):
                        msl = slice(m * 128, (m + 1) * 128)
                        psA = ps_bc.tile([128, CH], F32, name="psA", tag="bc")
                        nc.tensor.matmul(psA[:, :], g1p[:, msl], a16[:, :],
                                         start=True, stop=True)
                        psC = ps_bc.tile([128, CH], F32, name="psC", tag="bc")
                        nc.tensor.matmul(psC[:, :], g1n[:, msl], ct16[:, :],
                                         start=True, stop=True)
                        tmp = psqv.tile([128, CH], F16, name="lntmp", tag="sqv")
                        nc.vector.tensor_mul(tmp[:, :], o1p[m][:, :], psA[:, :])
                        nc.vector.scalar_tensor_tensor(
                            out=o1[m][:, :], in0=tmp[:, :], scalar=b1l[:, m:m + 1],
                            in1=psC[:, :], op0=OP.add, op1=OP.add)

                    # --- FFN1 + relu ---
                    for f in range(FT):
                        ps = ps_mm.tile([128, CH], F32, name="ffn1", tag="mm")
                        for k in range(DT):
                            nc.tensor.matmul(ps[:, :], w1[k][:, f * 128:(f + 1) * 128],
                                             o1[k][:, :], start=(k == 0),
                                             stop=(k == DT - 1))
                        nc.scalar.activation(out=h16[f][:, :], in_=ps[:, :],
                                             func=AF.Relu, bias=b1[:, f:f + 1])

                    # --- lnf stats (apply folded into w2g/vgn/bvb) ---
                    ln_stats(h16, DFF, "lnf", sq_act=True)
                    mu16 = pcr.tile([1, CH], F16, name="mu16", tag="crow")
                    nc.vector.tensor_copy(mu16, m_row)
                    af16 = pcr.tile([1, CH], F16, name="af16", tag="crow")
                    nc.vector.tensor_copy(af16, v_row)
                    ps_ab = ps_bc.tile([128, CH], F32, name="ab_ps", tag="bc")
                    nc.tensor.matmul(ps_ab[:, :], onesr[:, :], af16[:, :],
                                     start=True, stop=True)
                    ab32 = pe.tile([128, CH], F32, name="ab32", tag="evac")
                    nc.vector.tensor_copy(ab32[:, :], ps_ab[:, :])

                    # --- FFN2 (lnf folded) + residual -> pre2 ---
                    for m in range(DT):
                        msl = slice(m * 128, (m + 1) * 128)
                        ps = ps_mm.tile([128, CH], F32, name="ffn2", tag="mm")
                        for k in range(FT):
                            nc.tensor.matmul(ps[:, :], w2g[k][:, msl], h16[k][:, :],
                                             start=(k == 0), stop=False)
                        nc.tensor.matmul(ps[:, :], vgn[:, msl], mu16[:, :],
                                         start=False, stop=True)
                        t32 = pe.tile([128, CH], F32, name="t32", tag="evac")
                        nc.vector.tensor_mul(t32[:, :], ps[:, :], ab32[:, :])
                        nc.vector.scalar_tensor_tensor(
                            out=pre2[m][:, :], in0=t32[:, :], scalar=bvb[:, m:m + 1],
                            in1=o1[m][:, :], op0=OP.add, op1=OP.add)

                    # --- LN2 -> new16 ---
                    ln_stats(pre2, D, "ln2", sq_act=False)
                    a216 = pcr.tile([1, CH], F16, name="a216", tag="crow")
                    nc.vector.tensor_copy(a216, v_row)
                    c216 = pcr.tile([1, CH], F16, name="c216", tag="crow")
                    nc.vector.tensor_mul(c216, m_row, v_row)
                    for m in range(DT):
                        msl = slice(m * 128, (m + 1) * 128)
                        psA = ps_bc.tile([128, CH], F32, name="psA2", tag="bc")
                        nc.tensor.matmul(psA[:, :], g2p[:, msl], a216[:, :],
                                         start=True, stop=True)
                        psC = ps_bc.tile([128, CH], F32, name="psC2", tag="bc")
                        nc.tensor.matmul(psC[:, :], g2n[:, msl], c216[:, :],
                                         start=True, stop=True)
                        tmp = psqv.tile([128, CH], F16, name="ln2tmp", tag="sqv")
                        nc.vector.tensor_mul(tmp[:, :], pre2[m][:, :], psA[:, :])
                        nc.vector.scalar_tensor_tensor(
                            out=new16[m][:, :], in0=tmp[:, :], scalar=b2l[:, m:m + 1],
                            in1=psC[:, :], op0=OP.add, op1=OP.add)

                    # --- st += uw_bcast * (new - st) ---
                    ps_uw = ps_bc.tile([128, CH], F32, name="uwps", tag="bc")
                    nc.tensor.matmul(ps_uw[:, :], onesr[:, :], uw_row[:, sl],
                                     start=True, stop=True)
                    for m in range(DT):
                        diff = pe.tile([128, CH], F32, name="diff", tag="evac")
                        nc.vector.tensor_sub(diff[:, :], new16[m][:, :], st32[m][:, sl])
                        upd = pe.tile([128, CH], F32, name="upd", tag="evac")
                        nc.vector.tensor_mul(upd[:, :], diff[:, :], ps_uw[:, :])
                        nc.vector.tensor_add(st32[m][:, sl], st32[m][:, sl], upd[:, :])
                        nc.vector.tensor_copy(st16[m][:, sl], st32[m][:, sl])

            # ---------------- transpose st back to natural, store ----------------
            for s in range(ST):
                on32 = pe.tile([128, D], F32, name="on32", tag="evac")
                for d in range(DT):
                    ps = ps_bc.tile([128, 128], F16, name="tr_out", tag="bc")
                    nc.tensor.transpose(ps[:, :], st16[d][:, s * 128:(s + 1) * 128],
                                        eye[:, :])
                    nc.vector.tensor_copy(on32[:, d * 128:(d + 1) * 128], ps[:, :])
                nc.sync.dma_start(out=out_d[s * 128:(s + 1) * 128, :], in_=on32[:, :])

    split_excess_waits(nc)
    return nc


_NC_CACHE = {}


def _get_nc():
    if "nc" not in _NC_CACHE:
        _NC_CACHE["nc"] = _build()
    return _NC_CACHE["nc"]


def kernel(**inputs):
    f16, f32 = np.float16, np.float32
    get = lambda n: np.asarray(inputs[n], f32)
    x = get("x")
    wq, bq_, wk, bk_, wv, bv_ = (get(n) for n in ("wq", "bq", "wk", "bk", "wv", "bv"))
    wo, bo_, w1, b1_ = (get(n) for n in ("wo", "bo", "w1", "b1"))
    lnf_g, lnf_b, w2, b2_ = (get(n) for n in ("lnf_g", "lnf_b", "w2", "b2"))
    ln1_g, ln1_b, ln2_g, ln2_b = (get(n) for n in ("ln1_g", "ln1_b", "ln2_g", "ln2_b"))
    act_w, act_b = get("act_w"), get("act_b")

    col = lambda v: np.ascontiguousarray(v.reshape(-1, 128).T).astype(f32)  # [128, nt]
    w2g = (w2 * lnf_g[:, None])
    vg = w2g.sum(axis=0)                     # [D]
    vb = (w2 * lnf_b[:, None]).sum(axis=0)   # [D]
    eh = np.zeros((2, 128), f16)
    eh[0, :64] = 1.0
    eh[1, 64:] = 1.0

    parts16 = {
        "wq": wq.astype(f16), "wk": wk.astype(f16), "wv": wv.astype(f16),
        "wo": wo.astype(f16), "w1": w1.astype(f16), "w2g": w2g.astype(f16),
        "bvr": bv_.reshape(1, D).astype(f16),
        "vgn": (-vg).reshape(1, D).astype(f16),
        "g1p": ln1_g.reshape(1, D).astype(f16),
        "g1n": (-ln1_g).reshape(1, D).astype(f16),
        "g2p": ln2_g.reshape(1, D).astype(f16),
        "g2n": (-ln2_g).reshape(1, D).astype(f16),
        "actw": np.ascontiguousarray(act_w.reshape(DT, 128).T).astype(f16),
        "onesc": np.ones((128, 1), f16),
        "onesr": np.ones((1, 128), f16),
        "ehA": eh[0:1], "ehB": eh[1:2],
        "eye": np.eye(128, dtype=f16),
    }
    parts32 = {
        "bq": col(bq_), "bk": col(bk_), "bo": col(bo_), "b1": col(b1_),
        "bvb": col(vb + b2_), "b1l": col(ln1_b), "b2l": col(ln2_b),
        "actb": np.full((128, 1), float(np.ravel(act_b)[0]), f32),
    }
    for name, shp in SPEC16:
        assert parts16[name].shape == shp, (name, parts16[name].shape, shp)
    for name, shp in SPEC32:
        assert parts32[name].shape == shp, (name, parts32[name].shape, shp)
    pk16 = np.concatenate([np.ascontiguousarray(parts16[n]).ravel() for n, _ in SPEC16])
    pk32 = np.concatenate([np.ascontiguousarray(parts32[n]).ravel() for n, _ in SPEC32])
    common = {"pk16": pk16, "pk32": pk32}
    in_maps = [dict(common, x32=np.ascontiguousarray(x[i])) for i in range(N_CORES)]

    nc = _get_nc()
    try:
        res = run_bass_kernel_spmd(nc, in_maps, list(range(N_CORES)))
    except Exception:
        # transient NRT/axon failures (e.g. NRT_EXEC_UNIT_UNRECOVERABLE after a
        # wedged device) usually clear on retry
        res = run_bass_kernel_spmd(nc, in_maps, list(range(N_CORES)))
    return np.stack([res.results[i]["out32"] for i in range(N_CORES)]).astype(f32)


# revision 17
# speedup vs baseline: 74.1637x; 1.0264x over previous
"""ACT universal-transformer encoder (nn_Encoder_38165079392904) on 8 TRN2 cores.

Strategy: pure data-parallel over batch (B=8 -> 1 batch element per core, no
collectives). Per core, activations are kept in "transposed" layout
[feature_dim on partitions, sequence on free], so every GEMM is a natural
Trainium matmul (out = lhsT.T @ rhs, contracting over partitions):

  stT [D=512, S=1024]:  qT,kT = W.T @ stT (transposed); v = stT.T @ Wv (natural)
  logitsT[k,q] = kT_h.T @ qT_h    per head (K=dh=64, two heads share PE rows)
  attnT = exp(logitsT/8)          ACT engine, psum -> fp16
  sums  = ones.T @ attnT          matmul ones-trick (col-paired head pairs)
  ctxT  = v_h.T @ attnT           col-paired head pairs in one psum
  normalization / LN stat rows are broadcast across partitions with K=1
  matmuls; LayerNorm over dff is folded into w2 host-side (w2g = w2*lnf_g).
  ACT halting runs in fp32 on a [128, 8] tile-major layout; the update
  weight row is re-ordered to sequence order by an SBUF->SBUF DMA.

Matmul operands are fp16 (this toolchain's walrus allows only ONE semaphore
wait per ISA instruction; fp16 matmuls lower to LDWEIGHTS+MATMUL and get two
slots, while 4-byte fp32/fp32r matmuls are single self-loading instructions
and routinely fail codegen under Tile). PSUM accumulation is fp32; the
cross-step state st and halting probabilities are fp32.

A post-pass (split_excess_waits) hoists any wait beyond the per-instruction
budget into standalone EventSemaphore instructions on the same engine.
"""

import sys

sys.path.insert(0, "/opt/trn_rl_repo")

import numpy as np
import concourse.bass as bass
import concourse.tile as tile
from concourse import mybir
from concourse.bass_utils import run_bass_kernel_spmd

B, S, D, H, DFF, STEPS = 8, 1024, 512, 8, 2048, 6
DH = D // H          # 64
DT = D // 128        # 4 d-tiles
FT = DFF // 128      # 16 f-tiles
ST = S // 128        # 8 s-tiles
NC = 2               # s-chunks
CH = S // NC         # 512
EPS = 1e-6
THR = 1.0 - 0.01

F32 = mybir.dt.float32
F16 = mybir.dt.float16
AF = mybir.ActivationFunctionType
OP = mybir.AluOpType

N_CORES = 8

# parameter blob layouts (order shared by host packing and device loads)
SPEC16 = [
    ("wq", (D, D)), ("wk", (D, D)), ("wv", (D, D)), ("wo", (D, D)),
    ("w1", (D, DFF)), ("w2g", (DFF, D)),
    ("bvr", (1, D)), ("vgn", (1, D)),
    ("g1p", (1, D)), ("g1n", (1, D)), ("g2p", (1, D)), ("g2n", (1, D)),
    ("actw", (128, DT)),
    ("onesc", (128, 1)), ("onesr", (1, 128)),
    ("ehA", (1, 128)), ("ehB", (1, 128)), ("eye", (128, 128)),
]
SPEC32 = [
    ("bq", (128, DT)), ("bk", (128, DT)), ("bo", (128, DT)),
    ("b1", (128, FT)), ("bvb", (128, DT)), ("b1l", (128, DT)),
    ("b2l", (128, DT)), ("actb", (128, 1)),
]


def _offsets(spec):
    off, table = 0, {}
    for name, (r, c) in spec:
        table[name] = (off, (r, c))
        off += r * c
    return table, off


OFF16, TOT16 = _offsets(SPEC16)
OFF32, TOT32 = _offsets(SPEC32)



def split_excess_waits(nc):
    """Walrus codegen allows one sem-wait per ISA instruction (two for 2-byte
    matmuls via the LDWEIGHTS+MATMUL split). Hoist excess waits into
    standalone EventSemaphore instructions on the same engine, just before
    the instruction, preserving program order and semantics."""
    cnt = 0
    for fn in nc.m.functions:
        for bb in fn.blocks:
            il = bb.instructions
            i = 0
            while i < len(il):
                inst = il[i]
                si = inst.sync_info
                if si is not None and si.on_wait:
                    waits = list(si.on_wait)
                    cap = 1
                    if len(waits) > cap:
                        keep, extra = waits[-cap:], waits[:-cap]
                        for w in extra:
                            ni = mybir.InstEventSemaphore(
                                name=f"{inst.name}-xw{cnt}", ins=[], outs=[],
                                sync_info=mybir.SyncInfo(on_wait=[w], on_update=[]))
                            cnt += 1
                            ni.engine = inst.engine
                            il.insert(i, ni)
                            i += 1
                        inst.sync_info = mybir.SyncInfo(
                            on_wait=keep, on_update=list(si.on_update))
                i += 1
    return cnt


def _build(steps=STEPS):
    nc = bass.Bass()
    dp = lambda name, shape, dt=F16: nc.declare_dram_parameter(
        name, shape, dt, isOutput=False)

    x_d = dp("x32", [S, D], F32)
    pk16_d = dp("pk16", [TOT16], F16)
    pk32_d = dp("pk32", [TOT32], F32)
    out_d = nc.declare_dram_parameter("out32", [S, D], F32, isOutput=True)

    def src16(name, r0=0, rows=None):
        off, (R, C) = OFF16[name]
        rows = R if rows is None else rows
        return pk16_d[off + r0 * C: off + (r0 + rows) * C].rearrange(
            "(p f) -> p f", p=rows)

    def src32(name):
        off, (R, C) = OFF32[name]
        return pk32_d[off: off + R * C].rearrange("(p f) -> p f", p=R)
    uw_scr = nc.dram_tensor("uw_scratch", [128, ST], F16)

    with tile.TileContext(nc) as tc, nc.allow_low_precision(
            reason="fp16 operand pipeline by design; fp32 accumulation in PSUM"):
        with (
            tc.tile_pool(name="persist", bufs=1) as pp,
            tc.tile_pool(name="evac", bufs=6) as pe,        # f32 [128,CH] temps
            tc.tile_pool(name="sqa", bufs=6) as psqa,       # ACT-written fp16 squares
            tc.tile_pool(name="sqv", bufs=6) as psqv,       # DVE fp16 temps
            tc.tile_pool(name="attn", bufs=16) as pa,       # attnT fp16 [128,CH]
            tc.tile_pool(name="rtm", bufs=10) as prt,       # [128,8] halting temps
            tc.tile_pool(name="crow", bufs=4) as pcr,       # [1,CH] fp16 mm-rhs rows
            tc.tile_pool(name="ps_mm", bufs=4, space="PSUM") as ps_mm,
            tc.tile_pool(name="ps_bc", bufs=2, space="PSUM") as ps_bc,
            tc.tile_pool(name="ps_sm", bufs=2, space="PSUM") as ps_sm,
        ):
            # ---------------- persistent tiles + param load ----------------
            def load(name, shape, src, dt=F16):
                t = pp.tile(shape, dt, name=name)
                nc.sync.dma_start(out=t, in_=src)
                return t

            wq = [load(f"wq{k}", [128, D], src16("wq", k * 128, 128)) for k in range(DT)]
            wk = [load(f"wk{k}", [128, D], src16("wk", k * 128, 128)) for k in range(DT)]
            wv = [load(f"wv{k}", [128, D], src16("wv", k * 128, 128)) for k in range(DT)]
            wo = [load(f"wo{k}", [128, D], src16("wo", k * 128, 128)) for k in range(DT)]
            w1 = [load(f"w1{k}", [128, DFF], src16("w1", k * 128, 128)) for k in range(DT)]
            w2g = [load(f"w2g{k}", [128, D], src16("w2g", k * 128, 128)) for k in range(FT)]
            bq = load("bq", [128, DT], src32("bq"), F32)
            bk = load("bk", [128, DT], src32("bk"), F32)
            bo = load("bo", [128, DT], src32("bo"), F32)
            b1 = load("b1", [128, FT], src32("b1"), F32)
            bvb = load("bvb", [128, DT], src32("bvb"), F32)
            b1l = load("b1l", [128, DT], src32("b1l"), F32)
            b2l = load("b2l", [128, DT], src32("b2l"), F32)
            bvr = load("bvr", [1, D], src16("bvr"))
            vgn = load("vgn", [1, D], src16("vgn"))
            g1p = load("g1p", [1, D], src16("g1p"))
            g1n = load("g1n", [1, D], src16("g1n"))
            g2p = load("g2p", [1, D], src16("g2p"))
            g2n = load("g2n", [1, D], src16("g2n"))
            actw = load("actw", [128, DT], src16("actw"))
            actb = load("actb", [128, 1], src32("actb"), F32)
            onesc = load("onesc", [128, 1], src16("onesc"))
            onesr = load("onesr", [1, 128], src16("onesr"))
            ehA = load("ehA", [1, 128], src16("ehA"))
            ehB = load("ehB", [1, 128], src16("ehB"))
            eye = load("eye", [128, 128], src16("eye"))

            st32 = [pp.tile([128, S], F32, name=f"st32_{d}") for d in range(DT)]
            st16 = [pp.tile([128, S], F16, name=f"st16_{d}") for d in range(DT)]
            kT = [pp.tile([128, S], F16, name=f"kT{d}") for d in range(DT)]
            vn = [pp.tile([128, H * (DH + 1)], F16, name=f"vn{s}") for s in range(ST)]
            qT = [pp.tile([128, CH], F16, name=f"qT{d}") for d in range(DT)]
            ctx = [pp.tile([128, CH], F16, name=f"ctx{d}") for d in range(DT)]
            o1p = [pp.tile([128, CH], F16, name=f"o1p{d}") for d in range(DT)]
            o1 = [pp.tile([128, CH], F16, name=f"o1{d}") for d in range(DT)]
            pre2 = [pp.tile([128, CH], F16, name=f"pre2_{d}") for d in range(DT)]
            new16 = [pp.tile([128, CH], F16, name=f"new16_{d}") for d in range(DT)]
            h16 = [pp.tile([128, CH], F16, name=f"h{f}") for f in range(FT)]
            hp_tm = pp.tile([128, ST], F32, name="hp_tm")      # halting prob, tile-major
            uw_row = pp.tile([1, S], F16, name="uw_row")       # update weight, seq order
            bvb_b = pp.tile([128, D], F32, name="bvb_b")       # bcast of v-bias row
            # LN stat rows (all at partition base 0)
            m_row = pp.tile([1, CH], F32, name="m_row")
            m2_row = pp.tile([1, CH], F32, name="m2_row")
            v_row = pp.tile([1, CH], F32, name="v_row")        # var -> sd -> rstd

            nc.vector.memset(hp_tm, 0.0)
            for s in range(ST):
                nc.vector.memset(vn[s].rearrange("p (h u) -> p h u", u=DH + 1)[:, :, DH:], 1.0)
            eps_t = pp.tile([1, 1], F32, name="eps_t")
            nc.vector.memset(eps_t, EPS)

            # broadcast v-bias row to all 128 partitions once
            for c2 in range(2):
                ps = ps_bc.tile([128, 256], F32, name="bv_bc", tag="bc")
                nc.tensor.matmul(ps[:, :], onesr[:, :], bvr[:, c2 * 256:(c2 + 1) * 256],
                                 start=True, stop=True)
                nc.vector.tensor_copy(bvb_b[:, c2 * 256:(c2 + 1) * 256], ps[:, :])

            # ---------------- load x, transpose into stT ----------------
            for s in range(ST):
                xs32 = pe.tile([128, D], F32, name="xs32", tag="evac")
                nc.sync.dma_start(out=xs32, in_=x_d[s * 128:(s + 1) * 128, :])
                x16 = psqv.tile([128, D], F16, name="x16", tag="sqv")
                nc.vector.tensor_copy(x16[:, :], xs32[:, :])
                for d in range(DT):
                    ps = ps_bc.tile([128, 128], F16, name="tr_in", tag="bc")
                    nc.tensor.transpose(ps[:, :], x16[:, d * 128:(d + 1) * 128], eye[:, :])
                    nc.vector.tensor_copy(st32[d][:, s * 128:(s + 1) * 128], ps[:, :])
                    nc.vector.tensor_copy(st16[d][:, s * 128:(s + 1) * 128], ps[:, :])

            # helper: LN stats for c-local fp16 tiles (sq_engine: 'act'|'dve')
            # -> fills m_row (mean) and v_row (rstd); both at base 0.
            def ln_stats(tiles, n_feat, tag, sq_act):
                ps = ps_sm.tile([64, CH], F32, name=f"st_{tag}", tag="sm")
                nt = len(tiles)
                for i, t in enumerate(tiles):
                    nc.tensor.matmul(ps[0:1, :], onesc[:, :], t[:, :],
                                     start=(i == 0), stop=(i == nt - 1))
                for i, t in enumerate(tiles):
                    if sq_act:
                        sq = psqa.tile([128, CH], F16, name=f"sqa_{tag}", tag="sqa")
                        nc.scalar.activation(out=sq[:, :], in_=t[:, :], func=AF.Square)
                    else:
                        sq = psqv.tile([128, CH], F16, name=f"sqv_{tag}", tag="sqv")
                        nc.vector.tensor_mul(sq[:, :], t[:, :], t[:, :])
                    nc.tensor.matmul(ps[32:33, :], onesc[:, :], sq[:, :],
                                     start=(i == 0), stop=(i == nt - 1))
                nc.vector.tensor_scalar(m_row, ps[0:1, :], 1.0 / n_feat, None, op0=OP.mult)
                nc.vector.tensor_mul(m2_row, m_row, m_row)
                # var = sum(x^2)/n - mean^2
                nc.vector.scalar_tensor_tensor(
                    out=v_row, in0=ps[32:33, :], scalar=1.0 / n_feat, in1=m2_row,
                    op0=OP.mult, op1=OP.subtract)
                nc.scalar.activation(out=v_row, in_=v_row, func=AF.Sqrt, bias=eps_t[:, :])
                nc.vector.reciprocal(out=v_row, in_=v_row)   # rstd

            # ---------------- the ACT steps ----------------
            for step in range(steps):
                # --- [A] ponder prob + halting (tile-major [128, ST]) ---
                ps_p = ps_sm.tile([128, ST], F32, name="ps_p", tag="sm")
                for s in range(ST):
                    for k in range(DT):
                        nc.tensor.matmul(
                            ps_p[:, s:s + 1], st16[k][:, s * 128:(s + 1) * 128],
                            actw[:, k:k + 1], start=(k == 0), stop=(k == DT - 1))
                p_tm = prt.tile([128, ST], F32, name="p_tm")
                nc.scalar.activation(out=p_tm, in_=ps_p[:, :], func=AF.Sigmoid,
                                     bias=actb[:, :])
                s0 = prt.tile([128, ST], F32, name="s0")
                nc.vector.tensor_scalar(s0, hp_tm, 1.0, None, op0=OP.is_lt)
                ps0 = prt.tile([128, ST], F32, name="ps0")
                nc.vector.tensor_mul(ps0, p_tm, s0)
                cand = prt.tile([128, ST], F32, name="cand")
                nc.vector.tensor_add(cand, hp_tm, ps0)
                nh = prt.tile([128, ST], F32, name="nh")
                nc.vector.tensor_scalar(nh, cand, THR, None, op0=OP.is_gt)
                nc.vector.tensor_mul(nh, nh, s0)
                s1 = prt.tile([128, ST], F32, name="s1")
                nc.vector.tensor_scalar(s1, cand, THR, None, op0=OP.is_le)
                nc.vector.tensor_mul(s1, s1, s0)
                pst = prt.tile([128, ST], F32, name="pst")
                nc.vector.tensor_mul(pst, p_tm, s1)
                nc.vector.tensor_add(hp_tm, hp_tm, pst)
                hm1 = prt.tile([128, ST], F32, name="hm1")
                nc.vector.tensor_scalar(hm1, hp_tm, 1.0, None, op0=OP.subtract)
                rem = prt.tile([128, ST], F32, name="rem")
                nc.vector.tensor_mul(rem, nh, hm1)           # -nh*(1-hp)
                nc.vector.tensor_sub(hp_tm, hp_tm, rem)      # hp += nh*(1-hp)
                uw_tm = prt.tile([128, ST], F16, name="uw_tm")
                nc.vector.tensor_sub(uw_tm, pst, rem)        # p*still + nh*(1-hp)
                # reorder tile-major -> sequence order via DRAM bounce (the
                # two DMAs share one HWDGE FIFO ring, so they stay ordered)
                nc.sync.dma_start(out=uw_scr[:, :], in_=uw_tm[:, :])
                nc.sync.dma_start(out=uw_row[0:1, :],
                                  in_=uw_scr.rearrange("p j -> j p")[:, :])

                # --- [B] kT (transposed) and v (natural) for all s ---
                for m in range(DT):
                    for c in range(NC):
                        sl = slice(c * CH, (c + 1) * CH)
                        ps = ps_mm.tile([128, CH], F32, name="kproj", tag="mm")
                        for k in range(DT):
                            nc.tensor.matmul(
                                ps[:, :], wk[k][:, m * 128:(m + 1) * 128],
                                st16[k][:, sl], start=(k == 0), stop=(k == DT - 1))
                        nc.vector.tensor_scalar_add(kT[m][:, sl], ps[:, :],
                                                    bk[:, m:m + 1])
                for s in range(ST):
                    ssl = slice(s * 128, (s + 1) * 128)
                    ps = ps_mm.tile([128, D], F32, name="vproj", tag="mm")
                    for k in range(DT):
                        nc.tensor.matmul(ps[:, :], st16[k][:, ssl], wv[k][:, :],
                                         start=(k == 0), stop=(k == DT - 1))
                    nc.vector.tensor_add(
                        vn[s].rearrange("p (h u) -> p h u", u=DH + 1)[:, :, :DH],
                        ps[:, :].rearrange("p (h u) -> p h u", u=DH),
                        bvb_b[:, :].rearrange("p (h u) -> p h u", u=DH))

                # --- per s-chunk: q-proj, attention, FFN block, st update ---
                for c in range(NC):
                    sl = slice(c * CH, (c + 1) * CH)

                    # qT for this chunk
                    for m in range(DT):
                        ps = ps_mm.tile([128, CH], F32, name="qproj", tag="mm")
                        for k in range(DT):
                            nc.tensor.matmul(
                                ps[:, :], wq[k][:, m * 128:(m + 1) * 128],
                                st16[k][:, sl], start=(k == 0), stop=(k == DT - 1))
                        nc.vector.tensor_scalar_add(qT[m][:, :], ps[:, :],
                                                    bq[:, m:m + 1])

                    # attention, per head pair
                    for t in range(H // 2):
                        at = {}
                        for ha in (0, 1):
                            hsl = slice(ha * 64, (ha + 1) * 64)
                            for kt in range(ST):
                                ksl = slice(kt * 128, (kt + 1) * 128)
                                psl = ps_mm.tile([128, CH], F32, name="logits", tag="mm")
                                nc.tensor.matmul(psl[:, :], kT[t][hsl, ksl],
                                                 qT[t][hsl, :], start=True, stop=True)
                                a = pa.tile([128, CH], F16, name="attnT", tag="attn")
                                nc.scalar.activation(out=a[:, :], in_=psl[:, :],
                                                     func=AF.Exp, scale=0.125)
                                at[(ha, kt)] = a
                        for ha in (0, 1):
                            h = 2 * t + ha
                            vsl = slice(h * (DH + 1), (h + 1) * (DH + 1))
                            ps_c = ps_bc.tile([128, CH], F32, name="ctxps", tag="bc")
                            for kt in range(ST):
                                nc.tensor.matmul(ps_c[0:DH + 1, :], vn[kt][:, vsl],
                                                 at[(ha, kt)][:, :],
                                                 start=(kt == 0), stop=(kt == ST - 1))
                            rcpA = pcr.tile([1, CH], F16, name="rcpA", tag="crow")
                            nc.vector.reciprocal(out=rcpA, in_=ps_c[64:65, :])
                            ps_rb = ps_bc.tile([128, CH], F32, name="rbps", tag="bc")
                            nc.tensor.matmul(ps_rb[0:DH, :], ehA[:, 0:DH], rcpA[:, :],
                                             start=True, stop=True)
                            rb32 = pe.tile([128, CH], F32, name="rb32", tag="evac")
                            nc.vector.tensor_copy(rb32[0:DH, :], ps_rb[0:DH, :])
                            nc.vector.tensor_mul(ctx[t][ha * 64:(ha + 1) * 64, :],
                                                 ps_c[0:DH, :], rb32[0:DH, :])

                    # --- output projection + residual (o1p) ---
                    for m in range(DT):
                        ps = ps_mm.tile([128, CH], F32, name="oproj", tag="mm")
                        for k in range(DT):
                            nc.tensor.matmul(ps[:, :], wo[k][:, m * 128:(m + 1) * 128],
                                             ctx[k][:, :], start=(k == 0),
                                             stop=(k == DT - 1))
                        nc.vector.scalar_tensor_tensor(
                            out=o1p[m][:, :], in0=ps[:, :], scalar=bo[:, m:m + 1],
                            in1=st32[m][:, sl], op0=OP.add, op1=OP.add)

                    # --- LN1 ---
                    ln_stats(o1p, D, "ln1", sq_act=False)
                    a16 = pcr.tile([1, CH], F16, name="a16", tag="crow")
                    nc.vector.tensor_copy(a16, v_row)
                    ct16 = pcr.tile([1, CH], F16, name="ct16", tag="crow")
                    nc.vector.tensor_mul(ct16, m_row, v_row)   # mean*rstd
                    for m in range(DT):
                        msl = slice(m * 128, (m + 1) * 128)
                        psA = ps_bc.tile([128, CH], F32, name="psA", tag="bc")
                        nc.tensor.matmul(psA[:, :], g1p[:, msl], a16[:, :],
                                         start=True, stop=True)
                        psC = ps_bc.tile([128, CH], F32, name="psC", tag="bc")
                        nc.tensor.matmul(psC[:, :], g1n[:, msl], ct16[:, :],
                                         start=True, stop=True)
                        tmp = psqv.tile([128, CH], F16, name="lntmp", tag="sqv")
                        nc.vector.tensor_mul(tmp[:, :], o1p[m][:, :], psA[:, :])
                        nc.vector.scalar_tensor_tensor(
                            out=o1[m][:, :], in0=tmp[:, :], scalar=b1l[:, m:m + 1],
                            in1=psC[:, :], op0=OP.add, op1=OP.add)

                    # --- FFN1 + relu ---
                    for f in range(FT):
                        ps = ps_mm.tile([128, CH], F32, name="ffn1", tag="mm")
                        for k in range(DT):
                            nc.tensor.matmul(ps[:, :], w1[k][:, f * 128:(f + 1) * 128],
                                             o1[k][:, :], start=(k == 0),
                                             stop=(k == DT - 1))
                        nc.scalar.activation(out=h16[f][:, :], in_=ps[:, :],
                                             func=AF.Relu, bias=b1[:, f:f + 1])

                    # --- lnf stats (apply folded into w2g/vgn/bvb) ---
                    ln_stats(h16, DFF, "lnf", sq_act=True)
                    mu16 = pcr.tile([1, CH], F16, name="mu16", tag="crow")
                    nc.vector.tensor_copy(mu16, m_row)
                    af16 = pcr.tile([1, CH], F16, name="af16", tag="crow")
                    nc.vector.tensor_copy(af16, v_row)
                    ps_ab = ps_bc.tile([128, CH], F32, name="ab_ps", tag="bc")
                    nc.tensor.matmul(ps_ab[:, :], onesr[:, :], af16[:, :],
                                     start=True, stop=True)
                    ab32 = pe.tile([128, CH], F32, name="ab32", tag="evac")
                    nc.vector.tensor_copy(ab32[:, :], ps_ab[:, :])

                    # --- FFN2 (lnf folded) + residual -> pre2 ---
                    for m in range(DT):
                        msl = slice(m * 128, (m + 1) * 128)
                        ps = ps_mm.tile([128, CH], F32, name="ffn2", tag="mm")
                        for k in range(FT):
                            nc.tensor.matmul(ps[:, :], w2g[k][:, msl], h16[k][:, :],
                                             start=(k == 0), stop=False)
                        nc.tensor.matmul(ps[:, :], vgn[:, msl], mu16[:, :],
                                         start=False, stop=True)
                        t32 = pe.tile([128, CH], F32, name="t32", tag="evac")
                        nc.vector.tensor_mul(t32[:, :], ps[:, :], ab32[:, :])
                        nc.vector.scalar_tensor_tensor(
                            out=pre2[m][:, :], in0=t32[:, :], scalar=bvb[:, m:m + 1],
                            in1=o1[m][:, :], op0=OP.add, op1=OP.add)

                    # --- LN2 -> new16 ---
                    ln_stats(pre2, D, "ln2", sq_act=False)
                    a216 = pcr.tile([1, CH], F16, name="a216", tag="crow")
                    nc.vector.tensor_copy(a216, v_row)
                    c216 = pcr.tile([1, CH], F16, name="c216", tag="crow")
                    nc.vector.tensor_mul(c216, m_row, v_row)
                    for m in range(DT):
                        msl = slice(m * 128, (m + 1) * 128)
                        psA = ps_bc.tile([128, CH], F32, name="psA2", tag="bc")
                        nc.tensor.matmul(psA[:, :], g2p[:, msl], a216[:, :],
                                         start=True, stop=True)
                        psC = ps_bc.tile([128, CH], F32, name="psC2", tag="bc")
                        nc.tensor.matmul(psC[:, :], g2n[:, msl], c216[:, :],
                                         start=True, stop=True)
                        tmp = psqv.tile([128, CH], F16, name="ln2tmp", tag="sqv")
                        nc.vector.tensor_mul(tmp[:, :], pre2[m][:, :], psA[:, :])
                        nc.vector.scalar_tensor_tensor(
                            out=new16[m][:, :], in0=tmp[:, :], scalar=b2l[:, m:m + 1],
                            in1=psC[:, :], op0=OP.add, op1=OP.add)

                    # --- st += uw_bcast * (new - st) ---
                    ps_uw = ps_bc.tile([128, CH], F32, name="uwps", tag="bc")
                    nc.tensor.matmul(ps_uw[:, :], onesr[:, :], uw_row[:, sl],
                                     start=True, stop=True)
                    for m in range(DT):
                        diff = pe.tile([128, CH], F32, name="diff", tag="evac")
                        nc.vector.tensor_sub(diff[:, :], new16[m][:, :], st32[m][:, sl])
                        upd = pe.tile([128, CH], F32, name="upd", tag="evac")
                        nc.vector.tensor_mul(upd[:, :], diff[:, :], ps_uw[:, :])
                        nc.vector.tensor_add(st32[m][:, sl], st32[m][:, sl], upd[:, :])
                        nc.vector.tensor_copy(st16[m][:, sl], st32[m][:, sl])

            # ---------------- transpose st back to natural, store ----------------
            for s in range(ST):
                on32 = pe.tile([128, D], F32, name="on32", tag="evac")
                for d in range(DT):
                    ps = ps_bc.tile([128, 128], F16, name="tr_out", tag="bc")
                    nc.tensor.transpose(ps[:, :], st16[d][:, s * 128:(s + 1) * 128],
                                        eye[:, :])
                    nc.vector.tensor_copy(on32[:, d * 128:(d + 1) * 128], ps[:, :])
                nc.sync.dma_start(out=out_d[s * 128:(s + 1) * 128, :], in_=on32[:, :])

    split_excess_waits(nc)
    return nc


_NC_CACHE = {}


def _get_nc():
    if "nc" not in _NC_CACHE:
        _NC_CACHE["nc"] = _build()
    return _NC_CACHE["nc"]


def kernel(**inputs):
    f16, f32 = np.float16, np.float32
    get = lambda n: np.asarray(inputs[n], f32)
    x = get("x")
    wq, bq_, wk, bk_, wv, bv_ = (get(n) for n in ("wq", "bq", "wk", "bk", "wv", "bv"))
    wo, bo_, w1, b1_ = (get(n) for n in ("wo", "bo", "w1", "b1"))
    lnf_g, lnf_b, w2, b2_ = (get(n) for n in ("lnf_g", "lnf_b", "w2", "b2"))
    ln1_g, ln1_b, ln2_g, ln2_b = (get(n) for n in ("ln1_g", "ln1_b", "ln2_g", "ln2_b"))
    act_w, act_b = get("act_w"), get("act_b")

    col = lambda v: np.ascontiguousarray(v.reshape(-1, 128).T).astype(f32)  # [128, nt]
    w2g = (w2 * lnf_g[:, None])
    vg = w2g.sum(axis=0)                     # [D]
    vb = (w2 * lnf_b[:, None]).sum(axis=0)   # [D]
    eh = np.zeros((2, 128), f16)
    eh[0, :64] = 1.0
    eh[1, 64:] = 1.0

    parts16 = {
        "wq": wq.astype(f16), "wk": wk.astype(f16), "wv": wv.astype(f16),
        "wo": wo.astype(f16), "w1": w1.astype(f16), "w2g": w2g.astype(f16),
        "bvr": bv_.reshape(1, D).astype(f16),
        "vgn": (-vg).reshape(1, D).astype(f16),
        "g1p": ln1_g.reshape(1, D).astype(f16),
        "g1n": (-ln1_g).reshape(1, D).astype(f16),
        "g2p": ln2_g.reshape(1, D).astype(f16),
        "g2n": (-ln2_g).reshape(1, D).astype(f16),
        "actw": np.ascontiguousarray(act_w.reshape(DT, 128).T).astype(f16),
        "onesc": np.ones((128, 1), f16),
        "onesr": np.ones((1, 128), f16),
        "ehA": eh[0:1], "ehB": eh[1:2],
        "eye": np.eye(128, dtype=f16),
    }
    parts32 = {
        "bq": col(bq_), "bk": col(bk_), "bo": col(bo_), "b1": col(b1_),
        "bvb": col(vb + b2_), "b1l": col(ln1_b), "b2l": col(ln2_b),
        "actb": np.full((128, 1), float(np.ravel(act_b)[0]), f32),
    }
    for name, shp in SPEC16:
        assert parts16[name].shape == shp, (name, parts16[name].shape, shp)
    for name, shp in SPEC32:
        assert parts32[name].shape == shp, (name, parts32[name].shape, shp)
    pk16 = np.concatenate([np.ascontiguousarray(parts16[n]).ravel() for n, _ in SPEC16])
    pk32 = np.concatenate([np.ascontiguousarray(parts32[n]).ravel() for n, _ in SPEC32])
    common = {"pk16": pk16, "pk32": pk32}
    in_maps = [dict(common, x32=np.ascontiguousarray(x[i])) for i in range(N_CORES)]

    nc = _get_nc()
    res = None
    for attempt in range(3):
        try:
            res = run_bass_kernel_spmd(nc, in_maps, list(range(N_CORES)))
            break
        except Exception:
            # transient NRT/axon failures (e.g. NRT_EXEC_UNIT_UNRECOVERABLE
            # after a wedged device) usually clear on retry
            if attempt == 2:
                raise
            import time as _time
            _time.sleep(2.0)
    return np.stack([res.results[i]["out32"] for i in range(N_CORES)]).astype(f32)


# revision 19
# speedup vs baseline: 75.5026x; 1.0181x over previous
"""ACT universal-transformer encoder (nn_Encoder_38165079392904) on 8 TRN2 cores.

Strategy: pure data-parallel over batch (B=8 -> 1 batch element per core, no
collectives). Per core, activations are kept in "transposed" layout
[feature_dim on partitions, sequence on free], so every GEMM is a natural
Trainium matmul (out = lhsT.T @ rhs, contracting over partitions):

  stT [D=512, S=1024]:  qT,kT = W.T @ stT (transposed); v = stT.T @ Wv (natural)
  logitsT[k,q] = kT_h.T @ qT_h    per head (K=dh=64, two heads share PE rows)
  attnT = exp(logitsT/8)          ACT engine, psum -> fp16
  sums  = ones.T @ attnT          matmul ones-trick (col-paired head pairs)
  ctxT  = v_h.T @ attnT           col-paired head pairs in one psum
  normalization / LN stat rows are broadcast across partitions with K=1
  matmuls; LayerNorm over dff is folded into w2 host-side (w2g = w2*lnf_g).
  ACT halting runs in fp32 on a [128, 8] tile-major layout; the update
  weight row is re-ordered to sequence order by an SBUF->SBUF DMA.

Matmul operands are fp16 (this toolchain's walrus allows only ONE semaphore
wait per ISA instruction; fp16 matmuls lower to LDWEIGHTS+MATMUL and get two
slots, while 4-byte fp32/fp32r matmuls are single self-loading instructions
and routinely fail codegen under Tile). PSUM accumulation is fp32; the
cross-step state st and halting probabilities are fp32.

A post-pass (split_excess_waits) hoists any wait beyond the per-instruction
budget into standalone EventSemaphore instructions on the same engine.
"""

import sys

sys.path.insert(0, "/opt/trn_rl_repo")

import numpy as np
import concourse.bass as bass
import concourse.tile as tile
from concourse import mybir
from concourse.bass_utils import run_bass_kernel_spmd

B, S, D, H, DFF, STEPS = 8, 1024, 512, 8, 2048, 6
DH = D // H          # 64
DT = D // 128        # 4 d-tiles
FT = DFF // 128      # 16 f-tiles
ST = S // 128        # 8 s-tiles
NC = 2               # s-chunks
CH = S // NC         # 512
EPS = 1e-6
THR = 1.0 - 0.01

F32 = mybir.dt.float32
F16 = mybir.dt.float16
AF = mybir.ActivationFunctionType
OP = mybir.AluOpType

N_CORES = 8

# parameter blob layouts (order shared by host packing and device loads)
SPEC16 = [
    ("wq", (D, D)), ("wk", (D, D)), ("wv", (D, D)), ("wo", (D, D)),
    ("w1", (D, DFF)), ("w2g", (DFF, D)),
    ("bvr", (1, D)), ("vgn", (1, D)),
    ("g1p", (1, D)), ("g1n", (1, D)), ("g2p", (1, D)), ("g2n", (1, D)),
    ("actw", (128, DT)),
    ("onesc", (128, 1)), ("onesr", (1, 128)),
    ("ehA", (1, 128)), ("ehB", (1, 128)), ("eye", (128, 128)),
]
SPEC32 = [
    ("bq", (128, DT)), ("bk", (128, DT)), ("bo", (128, DT)),
    ("b1", (128, FT)), ("bvb", (128, DT)), ("b1l", (128, DT)),
    ("b2l", (128, DT)), ("actb", (128, 1)),
]


def _offsets(spec):
    off, table = 0, {}
    for name, (r, c) in spec:
        table[name] = (off, (r, c))
        off += r * c
    return table, off


OFF16, TOT16 = _offsets(SPEC16)
OFF32, TOT32 = _offsets(SPEC32)



def split_excess_waits(nc):
    """Walrus codegen allows one sem-wait per ISA instruction (two for 2-byte
    matmuls via the LDWEIGHTS+MATMUL split). Hoist excess waits into
    standalone EventSemaphore instructions on the same engine, just before
    the instruction, preserving program order and semantics."""
    cnt = 0
    for fn in nc.m.functions:
        for bb in fn.blocks:
            il = bb.instructions
            i = 0
            while i < len(il):
                inst = il[i]
                si = inst.sync_info
                if si is not None and si.on_wait:
                    waits = list(si.on_wait)
                    cap = 1
                    if len(waits) > cap:
                        keep, extra = waits[-cap:], waits[:-cap]
                        for w in extra:
                            ni = mybir.InstEventSemaphore(
                                name=f"{inst.name}-xw{cnt}", ins=[], outs=[],
                                sync_info=mybir.SyncInfo(on_wait=[w], on_update=[]))
                            cnt += 1
                            ni.engine = inst.engine
                            il.insert(i, ni)
                            i += 1
                        inst.sync_info = mybir.SyncInfo(
                            on_wait=keep, on_update=list(si.on_update))
                i += 1
    return cnt


def _build(steps=STEPS):
    nc = bass.Bass()
    dp = lambda name, shape, dt=F16: nc.declare_dram_parameter(
        name, shape, dt, isOutput=False)

    x_d = dp("x32", [S, D], F32)
    pk16_d = dp("pk16", [TOT16], F16)
    pk32_d = dp("pk32", [TOT32], F32)
    out_d = nc.declare_dram_parameter("out32", [S, D], F32, isOutput=True)

    def src16(name, r0=0, rows=None):
        off, (R, C) = OFF16[name]
        rows = R if rows is None else rows
        return pk16_d[off + r0 * C: off + (r0 + rows) * C].rearrange(
            "(p f) -> p f", p=rows)

    def src32(name):
        off, (R, C) = OFF32[name]
        return pk32_d[off: off + R * C].rearrange("(p f) -> p f", p=R)
    uw_scr = nc.dram_tensor("uw_scratch", [128, ST], F16)

    with tile.TileContext(nc) as tc, nc.allow_low_precision(
            reason="fp16 operand pipeline by design; fp32 accumulation in PSUM"):
        with (
            tc.tile_pool(name="persist", bufs=1) as pp,
            tc.tile_pool(name="evac", bufs=6) as pe,        # f32 [128,CH] temps
            tc.tile_pool(name="sqa", bufs=4) as psqa,       # ACT-written fp16 squares
            tc.tile_pool(name="sqv", bufs=5) as psqv,       # DVE fp16 temps
            tc.tile_pool(name="attn", bufs=24) as pa,       # attnT fp16 [128,CH]
            tc.tile_pool(name="rtm", bufs=10) as prt,       # [128,8] halting temps
            tc.tile_pool(name="crow", bufs=4) as pcr,       # [1,CH] fp16 mm-rhs rows
            tc.tile_pool(name="ps_mm", bufs=3, space="PSUM") as ps_mm,
            tc.tile_pool(name="ps_bc", bufs=4, space="PSUM") as ps_bc,
            tc.tile_pool(name="ps_sm", bufs=1, space="PSUM") as ps_sm,
        ):
            # ---------------- persistent tiles + param load ----------------
            def load(name, shape, src, dt=F16):
                t = pp.tile(shape, dt, name=name)
                nc.sync.dma_start(out=t, in_=src)
                return t

            wq = [load(f"wq{k}", [128, D], src16("wq", k * 128, 128)) for k in range(DT)]
            wk = [load(f"wk{k}", [128, D], src16("wk", k * 128, 128)) for k in range(DT)]
            wv = [load(f"wv{k}", [128, D], src16("wv", k * 128, 128)) for k in range(DT)]
            wo = [load(f"wo{k}", [128, D], src16("wo", k * 128, 128)) for k in range(DT)]
            w1 = [load(f"w1{k}", [128, DFF], src16("w1", k * 128, 128)) for k in range(DT)]
            w2g = [load(f"w2g{k}", [128, D], src16("w2g", k * 128, 128)) for k in range(FT)]
            bq = load("bq", [128, DT], src32("bq"), F32)
            bk = load("bk", [128, DT], src32("bk"), F32)
            bo = load("bo", [128, DT], src32("bo"), F32)
            b1 = load("b1", [128, FT], src32("b1"), F32)
            bvb = load("bvb", [128, DT], src32("bvb"), F32)
            b1l = load("b1l", [128, DT], src32("b1l"), F32)
            b2l = load("b2l", [128, DT], src32("b2l"), F32)
            bvr = load("bvr", [1, D], src16("bvr"))
            vgn = load("vgn", [1, D], src16("vgn"))
            g1p = load("g1p", [1, D], src16("g1p"))
            g1n = load("g1n", [1, D], src16("g1n"))
            g2p = load("g2p", [1, D], src16("g2p"))
            g2n = load("g2n", [1, D], src16("g2n"))
            actw = load("actw", [128, DT], src16("actw"))
            actb = load("actb", [128, 1], src32("actb"), F32)
            onesc = load("onesc", [128, 1], src16("onesc"))
            onesr = load("onesr", [1, 128], src16("onesr"))
            ehA = load("ehA", [1, 128], src16("ehA"))
            ehB = load("ehB", [1, 128], src16("ehB"))
            eye = load("eye", [128, 128], src16("eye"))

            st32 = [pp.tile([128, S], F32, name=f"st32_{d}") for d in range(DT)]
            st16 = [pp.tile([128, S], F16, name=f"st16_{d}") for d in range(DT)]
            kT = [pp.tile([128, S], F16, name=f"kT{d}") for d in range(DT)]
            vn = [pp.tile([128, H * (DH + 1)], F16, name=f"vn{s}") for s in range(ST)]
            qT = [pp.tile([128, CH], F16, name=f"qT{d}") for d in range(DT)]
            ctx = [pp.tile([128, CH], F16, name=f"ctx{d}") for d in range(DT)]
            o1p = [pp.tile([128, CH], F16, name=f"o1p{d}") for d in range(DT)]
            o1 = [pp.tile([128, CH], F16, name=f"o1{d}") for d in range(DT)]
            pre2 = [pp.tile([128, CH], F16, name=f"pre2_{d}") for d in range(DT)]
            new16 = [pp.tile([128, CH], F16, name=f"new16_{d}") for d in range(DT)]
            h16 = [pp.tile([128, CH], F16, name=f"h{f}") for f in range(FT)]
            hp_tm = pp.tile([128, ST], F32, name="hp_tm")      # halting prob, tile-major
            uw_row = pp.tile([1, S], F16, name="uw_row")       # update weight, seq order
            bvb_b = pp.tile([128, D], F32, name="bvb_b")       # bcast of v-bias row
            # LN stat rows (all at partition base 0)
            m_row = pp.tile([1, CH], F32, name="m_row")
            m2_row = pp.tile([1, CH], F32, name="m2_row")
            v_row = pp.tile([1, CH], F32, name="v_row")        # var -> sd -> rstd

            nc.vector.memset(hp_tm, 0.0)
            for s in range(ST):
                nc.vector.memset(vn[s].rearrange("p (h u) -> p h u", u=DH + 1)[:, :, DH:], 1.0)
            eps_t = pp.tile([1, 1], F32, name="eps_t")
            nc.vector.memset(eps_t, EPS)

            # broadcast v-bias row to all 128 partitions once
            for c2 in range(2):
                ps = ps_bc.tile([128, 256], F32, name="bv_bc", tag="bc")
                nc.tensor.matmul(ps[:, :], onesr[:, :], bvr[:, c2 * 256:(c2 + 1) * 256],
                                 start=True, stop=True)
                nc.vector.tensor_copy(bvb_b[:, c2 * 256:(c2 + 1) * 256], ps[:, :])

            # ---------------- load x, transpose into stT ----------------
            for s in range(ST):
                xs32 = pe.tile([128, D], F32, name="xs32", tag="evac")
                nc.sync.dma_start(out=xs32, in_=x_d[s * 128:(s + 1) * 128, :])
                x16 = psqv.tile([128, D], F16, name="x16", tag="sqv")
                nc.vector.tensor_copy(x16[:, :], xs32[:, :])
                for d in range(DT):
                    ps = ps_bc.tile([128, 128], F16, name="tr_in", tag="bc")
                    nc.tensor.transpose(ps[:, :], x16[:, d * 128:(d + 1) * 128], eye[:, :])
                    nc.vector.tensor_copy(st32[d][:, s * 128:(s + 1) * 128], ps[:, :])
                    nc.vector.tensor_copy(st16[d][:, s * 128:(s + 1) * 128], ps[:, :])

            # helper: LN stats for c-local fp16 tiles (sq_engine: 'act'|'dve')
            # -> fills m_row (mean) and v_row (rstd); both at base 0.
            def ln_stats(tiles, n_feat, tag, sq_act):
                ps = ps_sm.tile([64, CH], F32, name=f"st_{tag}", tag="sm")
                nt = len(tiles)
                for i, t in enumerate(tiles):
                    nc.tensor.matmul(ps[0:1, :], onesc[:, :], t[:, :],
                                     start=(i == 0), stop=(i == nt - 1))
                for i, t in enumerate(tiles):
                    if sq_act:
                        sq = psqa.tile([128, CH], F16, name=f"sqa_{tag}", tag="sqa")
                        nc.scalar.activation(out=sq[:, :], in_=t[:, :], func=AF.Square)
                    else:
                        sq = psqv.tile([128, CH], F16, name=f"sqv_{tag}", tag="sqv")
                        nc.vector.tensor_mul(sq[:, :], t[:, :], t[:, :])
                    nc.tensor.matmul(ps[32:33, :], onesc[:, :], sq[:, :],
                                     start=(i == 0), stop=(i == nt - 1))
                nc.vector.tensor_scalar(m_row, ps[0:1, :], 1.0 / n_feat, None, op0=OP.mult)
                nc.vector.tensor_mul(m2_row, m_row, m_row)
                # var = sum(x^2)/n - mean^2
                nc.vector.scalar_tensor_tensor(
                    out=v_row, in0=ps[32:33, :], scalar=1.0 / n_feat, in1=m2_row,
                    op0=OP.mult, op1=OP.subtract)
                nc.scalar.activation(out=v_row, in_=v_row, func=AF.Sqrt, bias=eps_t[:, :])
                nc.vector.reciprocal(out=v_row, in_=v_row)   # rstd

            # ---------------- the ACT steps ----------------
            for step in range(steps):
                # --- [A] ponder prob + halting (tile-major [128, ST]) ---
                ps_p = ps_sm.tile([128, ST], F32, name="ps_p", tag="sm")
                for s in range(ST):
                    for k in range(DT):
                        nc.tensor.matmul(
                            ps_p[:, s:s + 1], st16[k][:, s * 128:(s + 1) * 128],
                            actw[:, k:k + 1], start=(k == 0), stop=(k == DT - 1))
                p_tm = prt.tile([128, ST], F32, name="p_tm")
                nc.scalar.activation(out=p_tm, in_=ps_p[:, :], func=AF.Sigmoid,
                                     bias=actb[:, :])
                s0 = prt.tile([128, ST], F32, name="s0")
                nc.vector.tensor_scalar(s0, hp_tm, 1.0, None, op0=OP.is_lt)
                ps0 = prt.tile([128, ST], F32, name="ps0")
                nc.vector.tensor_mul(ps0, p_tm, s0)
                cand = prt.tile([128, ST], F32, name="cand")
                nc.vector.tensor_add(cand, hp_tm, ps0)
                nh = prt.tile([128, ST], F32, name="nh")
                nc.vector.tensor_scalar(nh, cand, THR, None, op0=OP.is_gt)
                nc.vector.tensor_mul(nh, nh, s0)
                s1 = prt.tile([128, ST], F32, name="s1")
                nc.vector.tensor_scalar(s1, cand, THR, None, op0=OP.is_le)
                nc.vector.tensor_mul(s1, s1, s0)
                pst = prt.tile([128, ST], F32, name="pst")
                nc.vector.tensor_mul(pst, p_tm, s1)
                nc.vector.tensor_add(hp_tm, hp_tm, pst)
                hm1 = prt.tile([128, ST], F32, name="hm1")
                nc.vector.tensor_scalar(hm1, hp_tm, 1.0, None, op0=OP.subtract)
                rem = prt.tile([128, ST], F32, name="rem")
                nc.vector.tensor_mul(rem, nh, hm1)           # -nh*(1-hp)
                nc.vector.tensor_sub(hp_tm, hp_tm, rem)      # hp += nh*(1-hp)
                uw_tm = prt.tile([128, ST], F16, name="uw_tm")
                nc.vector.tensor_sub(uw_tm, pst, rem)        # p*still + nh*(1-hp)
                # reorder tile-major -> sequence order via DRAM bounce (the
                # two DMAs share one HWDGE FIFO ring, so they stay ordered)
                nc.sync.dma_start(out=uw_scr[:, :], in_=uw_tm[:, :])
                nc.sync.dma_start(out=uw_row[0:1, :],
                                  in_=uw_scr.rearrange("p j -> j p")[:, :])

                # --- [B] kT (transposed) and v (natural) for all s ---
                for m in range(DT):
                    for c in range(NC):
                        sl = slice(c * CH, (c + 1) * CH)
                        ps = ps_mm.tile([128, CH], F32, name="kproj", tag="mm")
                        for k in range(DT):
                            nc.tensor.matmul(
                                ps[:, :], wk[k][:, m * 128:(m + 1) * 128],
                                st16[k][:, sl], start=(k == 0), stop=(k == DT - 1))
                        nc.vector.tensor_scalar_add(kT[m][:, sl], ps[:, :],
                                                    bk[:, m:m + 1])
                for s in range(ST):
                    ssl = slice(s * 128, (s + 1) * 128)
                    ps = ps_mm.tile([128, D], F32, name="vproj", tag="mm")
                    for k in range(DT):
                        nc.tensor.matmul(ps[:, :], st16[k][:, ssl], wv[k][:, :],
                                         start=(k == 0), stop=(k == DT - 1))
                    nc.vector.tensor_add(
                        vn[s].rearrange("p (h u) -> p h u", u=DH + 1)[:, :, :DH],
                        ps[:, :].rearrange("p (h u) -> p h u", u=DH),
                        bvb_b[:, :].rearrange("p (h u) -> p h u", u=DH))

                # --- per s-chunk: q-proj, attention, FFN block, st update ---
                for c in range(NC):
                    sl = slice(c * CH, (c + 1) * CH)

                    # qT for this chunk
                    for m in range(DT):
                        ps = ps_mm.tile([128, CH], F32, name="qproj", tag="mm")
                        for k in range(DT):
                            nc.tensor.matmul(
                                ps[:, :], wq[k][:, m * 128:(m + 1) * 128],
                                st16[k][:, sl], start=(k == 0), stop=(k == DT - 1))
                        nc.vector.tensor_scalar_add(qT[m][:, :], ps[:, :],
                                                    bq[:, m:m + 1])

                    # attention, per head pair
                    def attn_ctx(t, at):
                        ps_cs = []
                        for ha in (0, 1):
                            h = 2 * t + ha
                            vsl = slice(h * (DH + 1), (h + 1) * (DH + 1))
                            ps_c = ps_bc.tile([128, CH], F32, name="ctxps", tag="bc")
                            for kt in range(ST):
                                nc.tensor.matmul(ps_c[0:DH + 1, :], vn[kt][:, vsl],
                                                 at[(ha, kt)][:, :],
                                                 start=(kt == 0), stop=(kt == ST - 1))
                            ps_cs.append(ps_c)
                        rcps = []
                        for ha in (0, 1):
                            rcpA = pcr.tile([1, CH], F16, name="rcpA", tag="crow")
                            nc.vector.reciprocal(out=rcpA, in_=ps_cs[ha][64:65, :])
                            rcps.append(rcpA)
                        for ha in (0, 1):
                            ps_rb = ps_bc.tile([128, CH], F32, name="rbps", tag="bc")
                            nc.tensor.matmul(ps_rb[0:DH, :], ehA[:, 0:DH], rcps[ha][:, :],
                                             start=True, stop=True)
                            rb32 = pe.tile([128, CH], F32, name="rb32", tag="evac")
                            nc.vector.tensor_copy(rb32[0:DH, :], ps_rb[0:DH, :])
                            nc.vector.tensor_mul(ctx[t][ha * 64:(ha + 1) * 64, :],
                                                 ps_cs[ha][0:DH, :], rb32[0:DH, :])

                    prev = None  # (t, at) one-iteration software pipeline
                    for t in range(H // 2):
                        at = {}
                        for ha in (0, 1):
                            hsl = slice(ha * 64, (ha + 1) * 64)
                            for kt in range(ST):
                                ksl = slice(kt * 128, (kt + 1) * 128)
                                psl = ps_mm.tile([128, CH], F32, name="logits", tag="mm")
                                nc.tensor.matmul(psl[:, :], kT[t][hsl, ksl],
                                                 qT[t][hsl, :], start=True, stop=True)
                                a = pa.tile([128, CH], F16, name="attnT", tag="attn")
                                nc.scalar.activation(out=a[:, :], in_=psl[:, :],
                                                     func=AF.Exp, scale=0.125)
                                at[(ha, kt)] = a
                        if prev is not None:
                            attn_ctx(*prev)
                        prev = (t, at)
                    attn_ctx(*prev)

                    # --- output projection + residual (o1p) ---
                    for m in range(DT):
                        ps = ps_mm.tile([128, CH], F32, name="oproj", tag="mm")
                        for k in range(DT):
                            nc.tensor.matmul(ps[:, :], wo[k][:, m * 128:(m + 1) * 128],
                                             ctx[k][:, :], start=(k == 0),
                                             stop=(k == DT - 1))
                        nc.vector.scalar_tensor_tensor(
                            out=o1p[m][:, :], in0=ps[:, :], scalar=bo[:, m:m + 1],
                            in1=st32[m][:, sl], op0=OP.add, op1=OP.add)

                    # --- LN1 ---
                    ln_stats(o1p, D, "ln1", sq_act=False)
                    a16 = pcr.tile([1, CH], F16, name="a16", tag="crow")
                    nc.vector.tensor_copy(a16, v_row)
                    ct16 = pcr.tile([1, CH], F16, name="ct16", tag="crow")
                    nc.vector.tensor_mul(ct16, m_row, v_row)   # mean*rstd
                    for m in range(DT):
                        msl = slice(m * 128, (m + 1) * 128)
                        psA = ps_bc.tile([128, CH], F32, name="psA", tag="bc")
                        nc.tensor.matmul(psA[:, :], g1p[:, msl], a16[:, :],
                                         start=True, stop=True)
                        psC = ps_bc.tile([128, CH], F32, name="psC", tag="bc")
                        nc.tensor.matmul(psC[:, :], g1n[:, msl], ct16[:, :],
                                         start=True, stop=True)
                        tmp = psqv.tile([128, CH], F16, name="lntmp", tag="sqv")
                        nc.vector.tensor_mul(tmp[:, :], o1p[m][:, :], psA[:, :])
                        nc.vector.scalar_tensor_tensor(
                            out=o1[m][:, :], in0=tmp[:, :], scalar=b1l[:, m:m + 1],
                            in1=psC[:, :], op0=OP.add, op1=OP.add)

                    # --- FFN1 + relu ---
                    for f in range(FT):
                        ps = ps_mm.tile([128, CH], F32, name="ffn1", tag="mm")
                        for k in range(DT):
                            nc.tensor.matmul(ps[:, :], w1[k][:, f * 128:(f + 1) * 128],
                                             o1[k][:, :], start=(k == 0),
                                             stop=(k == DT - 1))
                        nc.scalar.activation(out=h16[f][:, :], in_=ps[:, :],
                                             func=AF.Relu, bias=b1[:, f:f + 1])

                    # --- lnf stats (apply folded into w2g/vgn/bvb) ---
                    ln_stats(h16, DFF, "lnf", sq_act=True)
                    mu16 = pcr.tile([1, CH], F16, name="mu16", tag="crow")
                    nc.vector.tensor_copy(mu16, m_row)
                    af16 = pcr.tile([1, CH], F16, name="af16", tag="crow")
                    nc.vector.tensor_copy(af16, v_row)
                    ps_ab = ps_bc.tile([128, CH], F32, name="ab_ps", tag="bc")
                    nc.tensor.matmul(ps_ab[:, :], onesr[:, :], af16[:, :],
                                     start=True, stop=True)
                    ab32 = pe.tile([128, CH], F32, name="ab32", tag="evac")
                    nc.vector.tensor_copy(ab32[:, :], ps_ab[:, :])

                    # --- FFN2 (lnf folded) + residual -> pre2 ---
                    for m in range(DT):
                        msl = slice(m * 128, (m + 1) * 128)
                        ps = ps_mm.tile([128, CH], F32, name="ffn2", tag="mm")
                        for k in range(FT):
                            nc.tensor.matmul(ps[:, :], w2g[k][:, msl], h16[k][:, :],
                                             start=(k == 0), stop=False)
                        nc.tensor.matmul(ps[:, :], vgn[:, msl], mu16[:, :],
                                         start=False, stop=True)
                        t32 = pe.tile([128, CH], F32, name="t32", tag="evac")
                        nc.vector.tensor_mul(t32[:, :], ps[:, :], ab32[:, :])
                        nc.vector.scalar_tensor_tensor(
                            out=pre2[m][:, :], in0=t32[:, :], scalar=bvb[:, m:m + 1],
                            in1=o1[m][:, :], op0=OP.add, op1=OP.add)

                    # --- LN2 -> new16 ---
                    ln_stats(pre2, D, "ln2", sq_act=False)
                    a216 = pcr.tile([1, CH], F16, name="a216", tag="crow")
                    nc.vector.tensor_copy(a216, v_row)
                    c216 = pcr.tile([1, CH], F16, name="c216", tag="crow")
                    nc.vector.tensor_mul(c216, m_row, v_row)
                    for m in range(DT):
                        msl = slice(m * 128, (m + 1) * 128)
                        psA = ps_bc.tile([128, CH], F32, name="psA2", tag="bc")
                        nc.tensor.matmul(psA[:, :], g2p[:, msl], a216[:, :],
                                         start=True, stop=True)
                        psC = ps_bc.tile([128, CH], F32, name="psC2", tag="bc")
                        nc.tensor.matmul(psC[:, :], g2n[:, msl], c216[:, :],
                                         start=True, stop=True)
                        tmp = psqv.tile([128, CH], F16, name="ln2tmp", tag="sqv")
                        nc.vector.tensor_mul(tmp[:, :], pre2[m][:, :], psA[:, :])
                        nc.vector.scalar_tensor_tensor(
                            out=new16[m][:, :], in0=tmp[:, :], scalar=b2l[:, m:m + 1],
                            in1=psC[:, :], op0=OP.add, op1=OP.add)

                    # --- st += uw_bcast * (new - st) ---
                    ps_uw = ps_bc.tile([128, CH], F32, name="uwps", tag="bc")
                    nc.tensor.matmul(ps_uw[:, :], onesr[:, :], uw_row[:, sl],
                                     start=True, stop=True)
                    for m in range(DT):
                        diff = pe.tile([128, CH], F32, name="diff", tag="evac")
                        nc.vector.tensor_sub(diff[:, :], new16[m][:, :], st32[m][:, sl])
                        upd = pe.tile([128, CH], F32, name="upd", tag="evac")
                        nc.vector.tensor_mul(upd[:, :], diff[:, :], ps_uw[:, :])
                        nc.vector.tensor_add(st32[m][:, sl], st32[m][:, sl], upd[:, :])
                        nc.vector.tensor_copy(st16[m][:, sl], st32[m][:, sl])

            # ---------------- transpose st back to natural, store ----------------
            for s in range(ST):
                on32 = pe.tile([128, D], F32, name="on32", tag="evac")
                for d in range(DT):
                    ps = ps_bc.tile([128, 128], F16, name="tr_out", tag="bc")
                    nc.tensor.transpose(ps[:, :], st16[d][:, s * 128:(s + 1) * 128],
                                        eye[:, :])
                    nc.vector.tensor_copy(on32[:, d * 128:(d + 1) * 128], ps[:, :])
                nc.sync.dma_start(out=out_d[s * 128:(s + 1) * 128, :], in_=on32[:, :])

    split_excess_waits(nc)
    return nc


_NC_CACHE = {}


def _get_nc():
    if "nc" not in _NC_CACHE:
        _NC_CACHE["nc"] = _build()
    return _NC_CACHE["nc"]


def kernel(**inputs):
    f16, f32 = np.float16, np.float32
    get = lambda n: np.asarray(inputs[n], f32)
    x = get("x")
    wq, bq_, wk, bk_, wv, bv_ = (get(n) for n in ("wq", "bq", "wk", "bk", "wv", "bv"))
    wo, bo_, w1, b1_ = (get(n) for n in ("wo", "bo", "w1", "b1"))
    lnf_g, lnf_b, w2, b2_ = (get(n) for n in ("lnf_g", "lnf_b", "w2", "b2"))
    ln1_g, ln1_b, ln2_g, ln2_b = (get(n) for n in ("ln1_g", "ln1_b", "ln2_g", "ln2_b"))
    act_w, act_b = get("act_w"), get("act_b")

    col = lambda v: np.ascontiguousarray(v.reshape(-1, 128).T).astype(f32)  # [128, nt]
    w2g = (w2 * lnf_g[:, None])
    vg = w2g.sum(axis=0)                     # [D]
    vb = (w2 * lnf_b[:, None]).sum(axis=0)   # [D]
    eh = np.zeros((2, 128), f16)
    eh[0, :64] = 1.0
    eh[1, 64:] = 1.0

    parts16 = {
        "wq": wq.astype(f16), "wk": wk.astype(f16), "wv": wv.astype(f16),
        "wo": wo.astype(f16), "w1": w1.astype(f16), "w2g": w2g.astype(f16),
        "bvr": bv_.reshape(1, D).astype(f16),
        "vgn": (-vg).reshape(1, D).astype(f16),
        "g1p": ln1_g.reshape(1, D).astype(f16),
        "g1n": (-ln1_g).reshape(1, D).astype(f16),
        "g2p": ln2_g.reshape(1, D).astype(f16),
        "g2n": (-ln2_g).reshape(1, D).astype(f16),
        "actw": np.ascontiguousarray(act_w.reshape(DT, 128).T).astype(f16),
        "onesc": np.ones((128, 1), f16),
        "onesr": np.ones((1, 128), f16),
        "ehA": eh[0:1], "ehB": eh[1:2],
        "eye": np.eye(128, dtype=f16),
    }
    parts32 = {
        "bq": col(bq_), "bk": col(bk_), "bo": col(bo_), "b1": col(b1_),
        "bvb": col(vb + b2_), "b1l": col(ln1_b), "b2l": col(ln2_b),
        "actb": np.full((128, 1), float(np.ravel(act_b)[0]), f32),
    }
    for name, shp in SPEC16:
        assert parts16[name].shape == shp, (name, parts16[name].shape, shp)
    for name, shp in SPEC32:
        assert parts32[name].shape == shp, (name, parts32[name].shape, shp)
    pk16 = np.concatenate([np.ascontiguousarray(parts16[n]).ravel() for n, _ in SPEC16])
    pk32 = np.concatenate([np.ascontiguousarray(parts32[n]).ravel() for n, _ in SPEC32])
    common = {"pk16": pk16, "pk32": pk32}
    in_maps = [dict(common, x32=np.ascontiguousarray(x[i])) for i in range(N_CORES)]

    nc = _get_nc()
    res = None
    for attempt in range(3):
        try:
            res = run_bass_kernel_spmd(nc, in_maps, list(range(N_CORES)))
            break
        except Exception:
            # transient NRT/axon failures (e.g. NRT_EXEC_UNIT_UNRECOVERABLE
            # after a wedged device) usually clear on retry
            if attempt == 2:
                raise
            import time as _time
            _time.sleep(2.0)
    return np.stack([res.results[i]["out32"] for i in range(N_CORES)]).astype(f32)
